# revision 24
# baseline (speedup 1.0000x reference)
"""Trainium2 Bass kernel for nn_Attention (sparse_attention, 8 NeuronCores).

Sharding: data-parallel over batch (4) x tensor-parallel over heads (2 groups
of 4 heads) = 8 cores. Each core computes attention for one batch and 4 heads
entirely in transposed (feature-major) layout, so no on-chip transposes are
needed. exp(attn_bias) is precomputed on the host in bf16, so on-chip softmax
is exp(S) * expB with no PSUM-blocking adds. Wo is row-sharded; each core
returns one bf16 partial per head-pair and the host reduces.

v3: all weights/x in bf16 (halves the weight DMA); DMA issue order tuned so
the first QK starts early and bias chunks stream per (pair, jt); for timing
loops the body is unrolled 2x with double-buffered weight/projection tiles so
iteration n+1's DMA head overlaps iteration n's compute tail.
"""

import os
import sys

for _p in ("/opt/trn_rl_repo", "/root/.axon_site/_ro/trn_rl_repo"):
    if os.path.isdir(_p) and _p not in sys.path:
        sys.path.append(_p)

import numpy as np

B, N, DIM, H, DH = 4, 1024, 512, 8, 64
SCALE = DH**-0.5
HL = 4  # heads per core
HDL = HL * DH  # 256 head-dims per core
NCORES = 8
NJT = N // 128  # 8 key-tiles
NKT = DIM // 128  # 4 contraction tiles

# wpack (bf16) column layout, ordered by first use
_WQK0 = 0  # wq_p0 4kt x 128 | wk_p0 4kt x 128
_XT = 1024  # 4 kt x 1024 tokens
_WV = 5120  # 4 kt x 256
_WQK1 = 6144  # wq_p1 | wk_p1
_WG = 7168  # 4 kt x 256
_WO = 8192  # 2 pair x 512
WPC = 9216

_CACHE = {}


def _build(loop_iters=1):
    import concourse.tile as tile
    from concourse import bacc, mybir

    fp32 = mybir.dt.float32
    f32r = mybir.dt.float32r
    bf16 = mybir.dt.bfloat16

    Exp = mybir.ActivationFunctionType.Exp
    Identity = mybir.ActivationFunctionType.Identity
    mult = mybir.AluOpType.mult

    nc = bacc.Bacc("TRN2", target_bir_lowering=False, debug=False, num_devices=NCORES)

    wpack = nc.dram_tensor("wpack", [128, WPC], bf16, kind="ExternalInput").ap()
    wsmall = nc.dram_tensor("wsmall", [128, 66], f32r, kind="ExternalInput").ap()
    expB = nc.dram_tensor("expB", [2, NJT, 128, 2 * N], bf16, kind="ExternalInput").ap()
    outT = nc.dram_tensor("outT", [2, 4, 128, N], bf16, kind="ExternalOutput").ap()

    from contextlib import ExitStack

    # unroll the loop body 2x so double-buffered tiles rotate across
    # iterations (a hardware loop reuses static SBUF addresses, so a single
    # body would serialize on its weight tiles)
    unroll = 2 if loop_iters > 1 else 1
    assert loop_iters % unroll == 0

    with tile.TileContext(nc) as tc, ExitStack() as stack:
        if loop_iters > 1:
            stack.enter_context(
                tc.For_i(0, loop_iters // unroll, 1, hint_engines=(mybir.EngineType.PE,))
            )
        with (
            tc.tile_pool(name="const", bufs=2) as cpool,
            tc.tile_pool(name="proj", bufs=2) as projpool,
            tc.tile_pool(name="bias", bufs=8) as biaspool,
            tc.tile_pool(name="etile", bufs=4) as epool,
            tc.tile_pool(name="work", bufs=2) as workpool,
            tc.tile_pool(name="psA", bufs=2, space="PSUM") as psA,
            tc.tile_pool(name="psB", bufs=2, space="PSUM") as psB,
        ):
            def body():
                # ---- SBUF homes for weights ----
                wp_sb = cpool.tile([128, WPC], bf16, tag="wp")
                ws_sb = cpool.tile([128, 66], f32r, tag="ws")
                bg_sb = ws_sb[:, 0:2]
                ones_sb = ws_sb[:, 2:66]

                def wq(p, kt):  # [128, 128] stationary for q proj of pair p
                    base = (_WQK0 if p == 0 else _WQK1) + kt * 128
                    return wp_sb[:, base : base + 128]

                def wk(p, kt):
                    base = (_WQK0 if p == 0 else _WQK1) + 512 + kt * 128
                    return wp_sb[:, base : base + 128]

                def xT(kt, lo, size):
                    return wp_sb[:, _XT + kt * 1024 + lo : _XT + kt * 1024 + lo + size]

                def wv(kt):
                    return wp_sb[:, _WV + kt * 256 : _WV + (kt + 1) * 256]

                def wg_(kt, mt):
                    base = _WG + kt * 256 + mt * 128
                    return wp_sb[:, base : base + 128]

                def wo_(p, mt):
                    base = _WO + p * 512 + mt * 128
                    return wp_sb[:, base : base + 128]

                # ---- DMA issue order (single sync ring => priority) ----
                nc.sync.dma_start(ws_sb[:], wsmall)
                # preload the Exp LUT off the critical path
                lutw = cpool.tile([1, 2], fp32, tag="lut")
                nc.scalar.activation(lutw[0:1, 0:1], ones_sb[0:1, 0:1], Exp)

                def wdma(lo, hi):
                    nc.sync.dma_start(wp_sb[:, lo:hi], wpack[:, lo:hi])

                bias_tiles = {}

                def bdma(p, jt):
                    bt = biaspool.tile([128, 2 * N], bf16, tag="bias", name=f"bt{p}_{jt}")
                    for hh in range(2):
                        nc.sync.dma_start(
                            bt[:, hh * N : (hh + 1) * N],
                            expB[p, jt, :, hh * N : (hh + 1) * N],
                        )
                    bias_tiles[(p, jt)] = bt

                wdma(_WQK0, _XT)  # wq_p0 | wk_p0
                for ih in range(2):  # ih-major half-chunks: q0's ih0 matmuls
                    for kt in range(NKT):  # start after half the xT stream
                        lo = _XT + kt * 1024 + ih * 512
                        wdma(lo, lo + 512)
                bdma(0, 0)
                wdma(_WV, _WQK1)
                bdma(0, 1)
                wdma(_WQK1, _WG)
                wdma(_WG, _WO)
                bdma(0, 2)
                wdma(_WO, WPC)
                for jt in range(3, NJT):
                    bdma(0, jt)
                for jt in range(NJT):
                    bdma(1, jt)

                # ---- projections ----
                qT_sb = [projpool.tile([128, N], bf16, tag=f"qT{m}", name=f"qT{m}") for m in range(2)]
                kT_sb = [projpool.tile([128, N], bf16, tag=f"kT{m}", name=f"kT{m}") for m in range(2)]
                gT_sb = [projpool.tile([128, N], fp32, tag=f"gT{m}", name=f"gT{m}") for m in range(2)]

                def projqk(which, p, evac_eng, split=None):
                    """q (which=0) or k (which=1) projection for pair p."""
                    wsel = wq if which == 0 else wk
                    dst = (qT_sb if which == 0 else kT_sb)[p]
                    ps = psA.tile([128, N], fp32, tag="big", name="ps")
                    for kt in range(NKT):
                        lhsT = wsel(p, kt)
                        for ih in range(2):
                            nc.tensor.matmul(
                                ps[:, ih * 512 : ih * 512 + 512],
                                lhsT,
                                xT(kt, ih * 512, 512),
                                start=(kt == 0),
                                stop=(kt == NKT - 1),
                            )
                    cp = nc.scalar.copy if evac_eng == "scalar" else nc.vector.tensor_copy
                    for lo, hi in split or [(0, N)]:
                        cp(dst[:, lo:hi], ps[:, lo:hi])

                def gproj(mt):
                    ps = psA.tile([128, N], fp32, tag="big", name="psg")
                    for kt in range(NKT):
                        lhsT = wg_(kt, mt)
                        for ih in range(2):
                            nc.tensor.matmul(
                                ps[:, ih * 512 : ih * 512 + 512],
                                lhsT,
                                xT(kt, ih * 512, 512),
                                start=(kt == 0),
                                stop=(kt == NKT - 1),
                            )
                    nc.scalar.activation(
                        gT_sb[mt][:], ps[:], Identity, bias=bg_sb[:, mt : mt + 1]
                    )

                # ---- v natural [token, d] + ones column per head (bf16) ----
                vhat_all = projpool.tile([128, NJT * HL * 65], bf16, tag="vhat")
                ones_view = vhat_all[:].rearrange(
                    "p (j h c) -> p j h c", j=NJT, c=65
                )[:, :, :, 64:65]
                nc.vector.memset(ones_view, 1.0)

                def vproj(jt):
                    vv = vhat_all[:, jt * HL * 65 : (jt + 1) * HL * 65].rearrange(
                        "p (h c) -> p h c", h=HL
                    )
                    ps2 = psA.tile([128, HDL], fp32, tag="big", name="ps2")
                    for kt in range(NKT):
                        nc.tensor.matmul(
                            ps2[:],
                            xT(kt, jt * 128, 128),
                            wv(kt),
                            start=(kt == 0),
                            stop=(kt == NKT - 1),
                        )
                    nc.vector.tensor_copy(
                        vv[:, :, 0:64], ps2[:].rearrange("p (h c) -> p h c", h=HL)
                    )

                # split evacs so the first QK (needs qT ih0 + kT cols 0:128)
                # unblocks as early as possible
                projqk(0, 0, "vector", split=[(0, 512), (512, N)])
                projqk(1, 0, "vector", split=[(0, 128), (128, N)])

                # ---- shared state across pairs ----
                U_sb = {}
                ug_sb = [
                    workpool.tile([128, N], bf16, tag=f"ug{p}", name=f"ug{p}", bufs=2)
                    for p in range(2)
                ]
                state = {}

                def attn_pair(p, background):
                    """jt-loop for head-pair p. AV matmuls run 3 (jt, hh)
                    units behind their QK so the in-order PE never waits on
                    the ACT-exp / DVE-mult chain; background thunks fill the
                    remaining PE slack (one slot per unit)."""
                    bgi = iter(background)
                    uv = [
                        psB.tile([65, N], fp32, tag="uv", name=f"uv{p}_{i}")
                        for i in range(2)
                    ]
                    pend = []

                    def flush_av():
                        jt0, hh0, e0 = pend.pop(0)
                        h = 2 * p + hh0
                        base = jt0 * HL * 65 + h * 65
                        for ih in range(2):
                            nc.tensor.matmul(
                                uv[hh0][:, ih * 512 : ih * 512 + 512],
                                vhat_all[:, base : base + 65],
                                e0[:, ih * 512 : ih * 512 + 512],
                                start=(jt0 == 0),
                                stop=(jt0 == NJT - 1),
                            )

                    for jt in range(NJT):
                        bt = bias_tiles[(p, jt)]
                        for hh in range(2):
                            st = psA.tile([128, N], fp32, tag="big", name=f"st{jt}_{hh}")
                            lhsT = kT_sb[p][hh * 64 : hh * 64 + 64, jt * 128 : jt * 128 + 128]
                            for ih in range(2):
                                nc.tensor.matmul(
                                    st[:, ih * 512 : ih * 512 + 512],
                                    lhsT,
                                    qT_sb[p][hh * 64 : hh * 64 + 64, ih * 512 : ih * 512 + 512],
                                    start=True,
                                    stop=True,
                                )
                            e1 = epool.tile([128, N], bf16, tag="e1", name="e1", bufs=3)
                            nc.scalar.activation(e1[:], st[:], Exp)
                            e = epool.tile([128, N], bf16, tag="e", name="e", bufs=5)
                            # a few pair-1 bias mults go to GPSIMD to relieve
                            # the DVE (SBUF-only operands qualify)
                            meng = (
                                nc.gpsimd
                                if (p == 1 and hh == 0 and jt % 2 == 1)
                                else nc.vector
                            )
                            meng.tensor_tensor(
                                out=e[:],
                                in0=e1[:],
                                in1=bt[:, hh * N : (hh + 1) * N],
                                op=mult,
                            )
                            pend.append((jt, hh, e))
                            if len(pend) > 3:
                                flush_av()
                            th = next(bgi, None)
                            if th is not None:
                                th()
                    for th in bgi:
                        if th is not None:
                            th()
                    return uv, pend, flush_av

                def epi_steps(p, hh, uv):
                    """Divide-by-denominator + gating for (p, hh). For (0,0)
                    the U*gT product runs on GPSIMD in parallel with the
                    reciprocal+broadcast chain (SBUF-SBUF TTs must share a
                    base partition, so only hh==0 qualifies). Elsewhere:
                    gs = broadcast(1/den)*gT (PSUM operand, exempt), then
                    ug = U*gs."""
                    par = (p, hh) == (0, 0)

                    def src():
                        return U_sb[(p, hh)] if p == 0 else uv[hh]

                    def s1():
                        rec = workpool.tile([1, N], f32r, tag="rec", name="rec", bufs=2)
                        with nc.allow_low_precision(reason="feeds PE broadcast"):
                            nc.vector.reciprocal(rec[:], src()[64:65, :])
                        state[("rec", p, hh)] = rec

                    def s1b():
                        ugp = workpool.tile([64, N], fp32, tag="gs", name="ugp", bufs=2)
                        nc.gpsimd.tensor_tensor(
                            out=ugp[:],
                            in0=src()[0:64, :],
                            in1=gT_sb[p][0:64, :],
                            op=mult,
                        )
                        state[("ugp", p, hh)] = ugp

                    def s2():
                        rec = state[("rec", p, hh)]
                        bc = psA.tile([64, N], fp32, tag="big", name="bc")
                        for ih in range(2):
                            nc.tensor.matmul(
                                bc[:, ih * 512 : ih * 512 + 512],
                                ones_sb[0:1, 0:64],
                                rec[0:1, ih * 512 : ih * 512 + 512],
                                start=True,
                                stop=True,
                            )
                        state[("bc", p, hh)] = bc
                        if not par:
                            gs = workpool.tile([64, N], fp32, tag="gs", name="gs", bufs=2)
                            nc.vector.tensor_tensor(
                                out=gs[:],
                                in0=bc[:],
                                in1=gT_sb[p][hh * 64 : hh * 64 + 64, :],
                                op=mult,
                            )
                            state[("gs", p, hh)] = gs

                    def s3():
                        if par:
                            nc.vector.tensor_tensor(
                                out=ug_sb[p][hh * 64 : hh * 64 + 64, :],
                                in0=state[("ugp", p, hh)][:],
                                in1=state[("bc", p, hh)][:],
                                op=mult,
                            )
                        else:
                            nc.vector.tensor_tensor(
                                out=ug_sb[p][hh * 64 : hh * 64 + 64, :],
                                in0=src()[0:64, :],
                                in1=state[("gs", p, hh)][:],
                                op=mult,
                            )

                    return ([s1, s1b, s2, s3] if par else [s1, s2, s3])

                def outproj_step(p, mt):
                    ps = psA.tile([128, N], fp32, tag="big", name="po")
                    lhsT = wo_(p, mt)
                    for ih in range(2):
                        nc.tensor.matmul(
                            ps[:, ih * 512 : ih * 512 + 512],
                            lhsT,
                            ug_sb[p][:, ih * 512 : ih * 512 + 512],
                            start=True,
                            stop=True,
                        )
                    ot = workpool.tile([128, N], bf16, tag="osb", name="osb", bufs=4)
                    if p == 1 or mt % 2 == 0:  # ACT is idle in the tail
                        nc.scalar.copy(ot[:], ps[:])
                    else:
                        nc.vector.tensor_copy(ot[:], ps[:])
                    nc.sync.dma_start(outT[p, mt], ot[:])

                def outproj_steps(p):
                    return [
                        (lambda p=p, mt=mt: outproj_step(p, mt)) for mt in range(4)
                    ]

                # ---- pair 0: backgrounds = v/g projections + pair-1 q/k ----
                bg0 = (
                    [lambda j=j: vproj(j) for j in range(NJT)]
                    + [
                        lambda: projqk(0, 1, "vector"),
                        lambda: projqk(1, 1, "vector"),
                        lambda: gproj(0),
                    ]
                )
                uv0, pend, flush = attn_pair(0, bg0)
                while pend:
                    flush()
                for hh in range(2):
                    U = workpool.tile([65, N], fp32, tag="U", name=f"U0_{hh}", bufs=4)
                    if hh == 0:
                        nc.scalar.copy(U[:], uv0[hh][:])
                    else:
                        nc.vector.tensor_copy(U[:], uv0[hh][:])
                    U_sb[(0, hh)] = U

                # ---- pair 1: backgrounds = pair-0 epilogue/outproj ----
                e00 = epi_steps(0, 0, uv0)
                e01 = epi_steps(0, 1, uv0)
                op0 = outproj_steps(0)
                bg1 = [lambda: gproj(1)] + e00 + [None] + e01 + [None] + op0
                uv1, pend, flush = attn_pair(1, bg1)
                # Tail: drain, then both heads' epilogues with maximal
                # engine parallelism.
                while pend:
                    flush()
                e10 = epi_steps(1, 0, uv1)
                e11 = epi_steps(1, 1, uv1)
                e10[0]()  # recip hh0 (DVE)
                e11[0]()  # recip hh1 (DVE)
                e10[1]()  # broadcast hh0 (PE) + gs hh0 (DVE)
                e11[1]()  # broadcast hh1 (PE) + gs hh1 (DVE)
                e10[2]()  # ug hh0 (DVE)
                e11[2]()  # ug hh1 (DVE)
                for th in outproj_steps(1):
                    th()

            for _ in range(unroll):
                body()

    nc.compile()
    return nc


def _shard_inputs(x, attn_bias, Wq, Wkv, Wg, bg, Wo):
    """Build per-core input maps (host-side layout prep)."""
    import ml_dtypes

    bf16 = ml_dtypes.bfloat16

    def kmaj(w):  # [512, F] -> [128, NKT*F] contraction-tile-major
        f = w.shape[1]
        return np.ascontiguousarray(
            w.reshape(NKT, 128, f).transpose(1, 0, 2)
        ).reshape(128, NKT * f)

    in_maps = []
    for d in range(NCORES):
        b, g = d // 2, d % 2
        cs = slice(g * HDL, (g + 1) * HDL)
        xTh = np.ascontiguousarray(x[b].T)  # [512, 1024]
        wq_all = Wq[:, cs] * SCALE  # [512, 256]
        wk_all = Wkv[:, g * HDL : (g + 1) * HDL]
        wv_all = Wkv[:, H * DH + g * HDL : H * DH + (g + 1) * HDL]
        wg_all = Wg[:, cs]
        chunks = [
            kmaj(wq_all[:, 0:128]),
            kmaj(wk_all[:, 0:128]),
            kmaj(xTh),
            kmaj(wv_all),
            kmaj(wq_all[:, 128:256]),
            kmaj(wk_all[:, 128:256]),
            kmaj(wg_all),
            np.ascontiguousarray(
                Wo[cs, :].reshape(2, 128, DIM).transpose(1, 0, 2).reshape(128, 2 * DIM)
            ),
        ]
        wpack = np.concatenate(chunks, axis=1).astype(bf16)
        wsmall = np.concatenate(
            [
                np.ascontiguousarray(bg[cs].reshape(2, 128).T),
                np.ones((128, 64), np.float32),
            ],
            axis=1,
        ).astype(np.float32)

        ab = attn_bias[b, g * HL : (g + 1) * HL]  # [4, N(i), N(j)]
        abT = ab.transpose(0, 2, 1).reshape(2, 2, NJT, 128, N)  # [p, hh, jt, jpart, i]
        eB = np.exp(abT.transpose(0, 2, 3, 1, 4))  # [p, jt, jpart, hh, i]
        expB = np.ascontiguousarray(eB).reshape(2, NJT, 128, 2 * N).astype(bf16)
        in_maps.append({"wpack": wpack, "wsmall": wsmall, "expB": expB})
    return in_maps


def _unshard(results, bo):
    out = np.empty((B, N, DIM), dtype=np.float32)
    for b in range(B):
        acc = results[2 * b]["outT"].astype(np.float32).sum(axis=0) + results[
            2 * b + 1
        ]["outT"].astype(np.float32).sum(axis=0)
        out[b] = acc.reshape(DIM, N).T + bo[None, :]
    return out


def kernel(x, mask, attn_bias, Wq, Wkv, Wg, bg, Wo, bo):
    """Full inputs in, full output out. mask is all-ones by construction."""
    from concourse.bass_utils import run_bass_kernel_spmd

    x = np.asarray(x, dtype=np.float32)
    attn_bias = np.asarray(attn_bias, dtype=np.float32)
    Wq = np.asarray(Wq, dtype=np.float32)
    Wkv = np.asarray(Wkv, dtype=np.float32)
    Wg = np.asarray(Wg, dtype=np.float32)
    bg = np.asarray(bg, dtype=np.float32)
    Wo = np.asarray(Wo, dtype=np.float32)
    bo = np.asarray(bo, dtype=np.float32)

    if "nc" not in _CACHE:
        _CACHE["nc"] = _build()
    in_maps = _shard_inputs(x, attn_bias, Wq, Wkv, Wg, bg, Wo)
    res = run_bass_kernel_spmd(_CACHE["nc"], in_maps, core_ids=list(range(NCORES)))
    return _unshard(res.results, bo)


# revision 25
# speedup vs baseline: 1.1385x; 1.1385x over previous
"""Trainium2 Bass kernel for nn_Attention (sparse_attention, 8 NeuronCores).

Sharding: data-parallel over batch (4) x tensor-parallel over heads (2 groups
of 4 heads) = 8 cores. Each core computes attention for one batch and 4 heads
entirely in transposed (feature-major) layout, so no on-chip transposes are
needed. exp(attn_bias) is precomputed on the host in bf16, so on-chip softmax
is exp(S) * expB with no PSUM-blocking adds. Wo is row-sharded; each core
returns one bf16 partial per head-pair and the host reduces.

v3: all weights/x in bf16 (halves the weight DMA); DMA issue order tuned so
the first QK starts early and bias chunks stream per (pair, jt); for timing
loops the body is unrolled 2x with double-buffered weight/projection tiles so
iteration n+1's DMA head overlaps iteration n's compute tail.
"""

import os
import sys

for _p in ("/opt/trn_rl_repo", "/root/.axon_site/_ro/trn_rl_repo"):
    if os.path.isdir(_p) and _p not in sys.path:
        sys.path.append(_p)

import numpy as np

B, N, DIM, H, DH = 4, 1024, 512, 8, 64
SCALE = DH**-0.5
HL = 4  # heads per core
HDL = HL * DH  # 256 head-dims per core
NCORES = 8
NJT = N // 128  # 8 key-tiles
NKT = DIM // 128  # 4 contraction tiles

# wpack (bf16) column layout, ordered by first use
_WQK0 = 0  # wq_p0 4kt x 128 | wk_p0 4kt x 128
_XT = 1024  # 4 kt x 1024 tokens
_WV = 5120  # 4 kt x 256
_WQK1 = 6144  # wq_p1 | wk_p1
_WG = 7168  # 4 kt x 256
_WO = 8192  # 2 pair x 512
WPC = 9216

_CACHE = {}


def _build(loop_iters=1):
    import concourse.tile as tile
    from concourse import bacc, mybir

    fp32 = mybir.dt.float32
    f32r = mybir.dt.float32r
    bf16 = mybir.dt.bfloat16

    Exp = mybir.ActivationFunctionType.Exp
    Identity = mybir.ActivationFunctionType.Identity
    mult = mybir.AluOpType.mult

    nc = bacc.Bacc("TRN2", target_bir_lowering=False, debug=False, num_devices=NCORES)

    wpack = nc.dram_tensor("wpack", [128, WPC], bf16, kind="ExternalInput").ap()
    wsmall = nc.dram_tensor("wsmall", [128, 66], f32r, kind="ExternalInput").ap()
    expB = nc.dram_tensor("expB", [2, NJT, 128, 2 * N], bf16, kind="ExternalInput").ap()
    outT = nc.dram_tensor("outT", [2, 4, 128, N], bf16, kind="ExternalOutput").ap()

    from contextlib import ExitStack

    with tile.TileContext(nc) as tc, ExitStack() as stack:
        if loop_iters > 1:
            stack.enter_context(
                tc.For_i(0, loop_iters, 1, hint_engines=(mybir.EngineType.PE,))
            )
        with (
            tc.tile_pool(name="const", bufs=1) as cpool,
            tc.tile_pool(name="proj", bufs=1) as projpool,
            tc.tile_pool(name="bias", bufs=8) as biaspool,
            tc.tile_pool(name="etile", bufs=4) as epool,
            tc.tile_pool(name="work", bufs=2) as workpool,
            tc.tile_pool(name="psA", bufs=2, space="PSUM") as psA,
            tc.tile_pool(name="psB", bufs=2, space="PSUM") as psB,
        ):
            def body():
                # ---- SBUF homes for weights ----
                wp_sb = cpool.tile([128, WPC], bf16, tag="wp")
                ws_sb = cpool.tile([128, 66], f32r, tag="ws")
                bg_sb = ws_sb[:, 0:2]
                ones_sb = ws_sb[:, 2:66]

                def wq(p, kt):  # [128, 128] stationary for q proj of pair p
                    base = (_WQK0 if p == 0 else _WQK1) + kt * 128
                    return wp_sb[:, base : base + 128]

                def wk(p, kt):
                    base = (_WQK0 if p == 0 else _WQK1) + 512 + kt * 128
                    return wp_sb[:, base : base + 128]

                def xT(kt, lo, size):
                    return wp_sb[:, _XT + kt * 1024 + lo : _XT + kt * 1024 + lo + size]

                def wv(kt):
                    return wp_sb[:, _WV + kt * 256 : _WV + (kt + 1) * 256]

                def wg_(kt, mt):
                    base = _WG + kt * 256 + mt * 128
                    return wp_sb[:, base : base + 128]

                def wo_(p, mt):
                    base = _WO + p * 512 + mt * 128
                    return wp_sb[:, base : base + 128]

                # ---- DMA issue order (single sync ring => priority) ----
                nc.sync.dma_start(ws_sb[:], wsmall)
                # PE p-state prewarm: dummy matmuls off a memset tile keep
                # the PE busy through the DMA head so the projections run at
                # full clock; the Exp LUT preload rides the same window.
                wtile = cpool.tile([1, 512], bf16, tag="wtile")
                nc.vector.memset(wtile[:], 0.5)
                for _ in range(11):
                    pw = psA.tile([1, 512], fp32, tag="big", name="pw")
                    nc.tensor.matmul(pw[:], wtile[0:1, 0:1], wtile[:], start=True, stop=True)
                lutw = cpool.tile([1, 2], fp32, tag="lut")
                nc.scalar.activation(lutw[0:1, 0:1], ones_sb[0:1, 0:1], Exp)

                def wdma(lo, hi):
                    nc.sync.dma_start(wp_sb[:, lo:hi], wpack[:, lo:hi])

                bias_tiles = {}

                def bdma(p, jt):
                    bt = biaspool.tile([128, 2 * N], bf16, tag="bias", name=f"bt{p}_{jt}")
                    for hh in range(2):
                        nc.sync.dma_start(
                            bt[:, hh * N : (hh + 1) * N],
                            expB[p, jt, :, hh * N : (hh + 1) * N],
                        )
                    bias_tiles[(p, jt)] = bt

                wdma(_WQK0, _XT)  # wq_p0 | wk_p0
                for ih in range(2):  # ih-major half-chunks: q0's ih0 matmuls
                    for kt in range(NKT):  # start after half the xT stream
                        lo = _XT + kt * 1024 + ih * 512
                        wdma(lo, lo + 512)
                bdma(0, 0)
                wdma(_WV, _WQK1)
                bdma(0, 1)
                wdma(_WQK1, _WG)
                wdma(_WG, _WO)
                bdma(0, 2)
                wdma(_WO, WPC)
                for jt in range(3, NJT):
                    bdma(0, jt)
                for jt in range(NJT):
                    bdma(1, jt)

                # ---- projections ----
                qT_sb = [projpool.tile([128, N], bf16, tag=f"qT{m}", name=f"qT{m}") for m in range(2)]
                kT_sb = [projpool.tile([128, N], bf16, tag=f"kT{m}", name=f"kT{m}") for m in range(2)]
                gT_sb = [projpool.tile([128, N], fp32, tag=f"gT{m}", name=f"gT{m}") for m in range(2)]

                def projqk(which, p, evac_eng, split=None):
                    """q (which=0) or k (which=1) projection for pair p."""
                    wsel = wq if which == 0 else wk
                    dst = (qT_sb if which == 0 else kT_sb)[p]
                    ps = psA.tile([128, N], fp32, tag="big", name="ps")
                    for kt in range(NKT):
                        lhsT = wsel(p, kt)
                        for ih in range(2):
                            nc.tensor.matmul(
                                ps[:, ih * 512 : ih * 512 + 512],
                                lhsT,
                                xT(kt, ih * 512, 512),
                                start=(kt == 0),
                                stop=(kt == NKT - 1),
                            )
                    cp = nc.scalar.copy if evac_eng == "scalar" else nc.vector.tensor_copy
                    for lo, hi in split or [(0, N)]:
                        cp(dst[:, lo:hi], ps[:, lo:hi])

                def gproj(mt):
                    ps = psA.tile([128, N], fp32, tag="big", name="psg")
                    for kt in range(NKT):
                        lhsT = wg_(kt, mt)
                        for ih in range(2):
                            nc.tensor.matmul(
                                ps[:, ih * 512 : ih * 512 + 512],
                                lhsT,
                                xT(kt, ih * 512, 512),
                                start=(kt == 0),
                                stop=(kt == NKT - 1),
                            )
                    nc.scalar.activation(
                        gT_sb[mt][:], ps[:], Identity, bias=bg_sb[:, mt : mt + 1]
                    )

                # ---- v natural [token, d] + ones column per head (bf16) ----
                vhat_all = projpool.tile([128, NJT * HL * 65], bf16, tag="vhat")
                ones_view = vhat_all[:].rearrange(
                    "p (j h c) -> p j h c", j=NJT, c=65
                )[:, :, :, 64:65]
                nc.vector.memset(ones_view, 1.0)

                def vproj(jt):
                    vv = vhat_all[:, jt * HL * 65 : (jt + 1) * HL * 65].rearrange(
                        "p (h c) -> p h c", h=HL
                    )
                    ps2 = psA.tile([128, HDL], fp32, tag="big", name="ps2")
                    for kt in range(NKT):
                        nc.tensor.matmul(
                            ps2[:],
                            xT(kt, jt * 128, 128),
                            wv(kt),
                            start=(kt == 0),
                            stop=(kt == NKT - 1),
                        )
                    nc.vector.tensor_copy(
                        vv[:, :, 0:64], ps2[:].rearrange("p (h c) -> p h c", h=HL)
                    )

                # split evacs so the first QK (needs qT ih0 + kT cols 0:128)
                # unblocks as early as possible
                projqk(0, 0, "vector", split=[(0, 512), (512, N)])
                projqk(1, 0, "vector", split=[(0, 128), (128, N)])

                # ---- shared state across pairs ----
                U_sb = {}
                ug_sb = [
                    workpool.tile([128, N], bf16, tag=f"ug{p}", name=f"ug{p}", bufs=2)
                    for p in range(2)
                ]
                state = {}

                def attn_pair(p, background):
                    """jt-loop for head-pair p. AV matmuls run 3 (jt, hh)
                    units behind their QK so the in-order PE never waits on
                    the ACT-exp / DVE-mult chain; background thunks fill the
                    remaining PE slack (one slot per unit)."""
                    bgi = iter(background)
                    uv = [
                        psB.tile([65, N], fp32, tag="uv", name=f"uv{p}_{i}")
                        for i in range(2)
                    ]
                    pend = []

                    def flush_av():
                        jt0, hh0, e0 = pend.pop(0)
                        h = 2 * p + hh0
                        base = jt0 * HL * 65 + h * 65
                        for ih in range(2):
                            nc.tensor.matmul(
                                uv[hh0][:, ih * 512 : ih * 512 + 512],
                                vhat_all[:, base : base + 65],
                                e0[:, ih * 512 : ih * 512 + 512],
                                start=(jt0 == 0),
                                stop=(jt0 == NJT - 1),
                            )

                    for jt in range(NJT):
                        bt = bias_tiles[(p, jt)]
                        for hh in range(2):
                            st = psA.tile([128, N], fp32, tag="big", name=f"st{jt}_{hh}")
                            lhsT = kT_sb[p][hh * 64 : hh * 64 + 64, jt * 128 : jt * 128 + 128]
                            for ih in range(2):
                                nc.tensor.matmul(
                                    st[:, ih * 512 : ih * 512 + 512],
                                    lhsT,
                                    qT_sb[p][hh * 64 : hh * 64 + 64, ih * 512 : ih * 512 + 512],
                                    start=True,
                                    stop=True,
                                )
                            e1 = epool.tile([128, N], bf16, tag="e1", name="e1", bufs=3)
                            nc.scalar.activation(e1[:], st[:], Exp)
                            e = epool.tile([128, N], bf16, tag="e", name="e", bufs=5)
                            nc.vector.tensor_tensor(
                                out=e[:],
                                in0=e1[:],
                                in1=bt[:, hh * N : (hh + 1) * N],
                                op=mult,
                            )
                            pend.append((jt, hh, e))
                            if len(pend) > 3:
                                flush_av()
                            th = next(bgi, None)
                            if th is not None:
                                th()
                    for th in bgi:
                        if th is not None:
                            th()
                    return uv, pend, flush_av

                def epi_steps(p, hh, uv):
                    """Divide-by-denominator + gating for (p, hh). For (0,0)
                    the U*gT product runs on GPSIMD in parallel with the
                    reciprocal+broadcast chain (SBUF-SBUF TTs must share a
                    base partition, so only hh==0 qualifies). Elsewhere:
                    gs = broadcast(1/den)*gT (PSUM operand, exempt), then
                    ug = U*gs."""
                    par = (p, hh) == (0, 0)

                    def src():
                        return U_sb[(p, hh)] if p == 0 else uv[hh]

                    def s1():
                        rec = workpool.tile([1, N], f32r, tag="rec", name="rec", bufs=2)
                        with nc.allow_low_precision(reason="feeds PE broadcast"):
                            nc.vector.reciprocal(rec[:], src()[64:65, :])
                        state[("rec", p, hh)] = rec

                    def s1b():
                        ugp = workpool.tile([64, N], fp32, tag="gs", name="ugp", bufs=2)
                        nc.gpsimd.tensor_tensor(
                            out=ugp[:],
                            in0=src()[0:64, :],
                            in1=gT_sb[p][0:64, :],
                            op=mult,
                        )
                        state[("ugp", p, hh)] = ugp

                    def s2():
                        rec = state[("rec", p, hh)]
                        bc = psA.tile([64, N], fp32, tag="big", name="bc")
                        for ih in range(2):
                            nc.tensor.matmul(
                                bc[:, ih * 512 : ih * 512 + 512],
                                ones_sb[0:1, 0:64],
                                rec[0:1, ih * 512 : ih * 512 + 512],
                                start=True,
                                stop=True,
                            )
                        state[("bc", p, hh)] = bc
                        if not par:
                            gs = workpool.tile([64, N], fp32, tag="gs", name="gs", bufs=2)
                            nc.vector.tensor_tensor(
                                out=gs[:],
                                in0=bc[:],
                                in1=gT_sb[p][hh * 64 : hh * 64 + 64, :],
                                op=mult,
                            )
                            state[("gs", p, hh)] = gs

                    def s3():
                        if par:
                            nc.vector.tensor_tensor(
                                out=ug_sb[p][hh * 64 : hh * 64 + 64, :],
                                in0=state[("ugp", p, hh)][:],
                                in1=state[("bc", p, hh)][:],
                                op=mult,
                            )
                        else:
                            nc.vector.tensor_tensor(
                                out=ug_sb[p][hh * 64 : hh * 64 + 64, :],
                                in0=src()[0:64, :],
                                in1=state[("gs", p, hh)][:],
                                op=mult,
                            )

                    return ([s1, s1b, s2, s3] if par else [s1, s2, s3])

                def outproj_step(p, mt):
                    ps = psA.tile([128, N], fp32, tag="big", name="po")
                    lhsT = wo_(p, mt)
                    for ih in range(2):
                        nc.tensor.matmul(
                            ps[:, ih * 512 : ih * 512 + 512],
                            lhsT,
                            ug_sb[p][:, ih * 512 : ih * 512 + 512],
                            start=True,
                            stop=True,
                        )
                    ot = workpool.tile([128, N], bf16, tag="osb", name="osb", bufs=4)
                    if p == 1 or mt % 2 == 0:  # ACT is idle in the tail
                        nc.scalar.copy(ot[:], ps[:])
                    else:
                        nc.vector.tensor_copy(ot[:], ps[:])
                    nc.sync.dma_start(outT[p, mt], ot[:])

                def outproj_steps(p):
                    return [
                        (lambda p=p, mt=mt: outproj_step(p, mt)) for mt in range(4)
                    ]

                # ---- pair 0: backgrounds = v/g projections + pair-1 q/k ----
                bg0 = (
                    [lambda j=j: vproj(j) for j in range(NJT)]
                    + [
                        lambda: projqk(0, 1, "vector"),
                        lambda: projqk(1, 1, "vector"),
                        lambda: gproj(0),
                    ]
                )
                uv0, pend, flush = attn_pair(0, bg0)
                while pend:
                    flush()
                for hh in range(2):
                    U = workpool.tile([65, N], fp32, tag="U", name=f"U0_{hh}", bufs=4)
                    if hh == 0:
                        nc.scalar.copy(U[:], uv0[hh][:])
                    else:
                        nc.vector.tensor_copy(U[:], uv0[hh][:])
                    U_sb[(0, hh)] = U

                # ---- pair 1: backgrounds = pair-0 epilogue/outproj ----
                e00 = epi_steps(0, 0, uv0)
                e01 = epi_steps(0, 1, uv0)
                op0 = outproj_steps(0)
                bg1 = [lambda: gproj(1)] + e00 + [None] + e01 + [None] + op0
                uv1, pend, flush = attn_pair(1, bg1)
                # Tail: drain, then both heads' epilogues with maximal
                # engine parallelism.
                while pend:
                    flush()
                e10 = epi_steps(1, 0, uv1)
                e11 = epi_steps(1, 1, uv1)
                e10[0]()  # recip hh0 (DVE)
                e11[0]()  # recip hh1 (DVE)
                e10[1]()  # broadcast hh0 (PE) + gs hh0 (DVE)
                e11[1]()  # broadcast hh1 (PE) + gs hh1 (DVE)
                e10[2]()  # ug hh0 (DVE)
                e11[2]()  # ug hh1 (DVE)
                for th in outproj_steps(1):
                    th()

            body()

    nc.compile()
    return nc


def _shard_inputs(x, attn_bias, Wq, Wkv, Wg, bg, Wo):
    """Build per-core input maps (host-side layout prep)."""
    import ml_dtypes

    bf16 = ml_dtypes.bfloat16

    def kmaj(w):  # [512, F] -> [128, NKT*F] contraction-tile-major
        f = w.shape[1]
        return np.ascontiguousarray(
            w.reshape(NKT, 128, f).transpose(1, 0, 2)
        ).reshape(128, NKT * f)

    in_maps = []
    for d in range(NCORES):
        b, g = d // 2, d % 2
        cs = slice(g * HDL, (g + 1) * HDL)
        xTh = np.ascontiguousarray(x[b].T)  # [512, 1024]
        wq_all = Wq[:, cs] * SCALE  # [512, 256]
        wk_all = Wkv[:, g * HDL : (g + 1) * HDL]
        wv_all = Wkv[:, H * DH + g * HDL : H * DH + (g + 1) * HDL]
        wg_all = Wg[:, cs]
        chunks = [
            kmaj(wq_all[:, 0:128]),
            kmaj(wk_all[:, 0:128]),
            kmaj(xTh),
            kmaj(wv_all),
            kmaj(wq_all[:, 128:256]),
            kmaj(wk_all[:, 128:256]),
            kmaj(wg_all),
            np.ascontiguousarray(
                Wo[cs, :].reshape(2, 128, DIM).transpose(1, 0, 2).reshape(128, 2 * DIM)
            ),
        ]
        wpack = np.concatenate(chunks, axis=1).astype(bf16)
        wsmall = np.concatenate(
            [
                np.ascontiguousarray(bg[cs].reshape(2, 128).T),
                np.ones((128, 64), np.float32),
            ],
            axis=1,
        ).astype(np.float32)

        ab = attn_bias[b, g * HL : (g + 1) * HL]  # [4, N(i), N(j)]
        abT = ab.transpose(0, 2, 1).reshape(2, 2, NJT, 128, N)  # [p, hh, jt, jpart, i]
        eB = np.exp(abT.transpose(0, 2, 3, 1, 4))  # [p, jt, jpart, hh, i]
        expB = np.ascontiguousarray(eB).reshape(2, NJT, 128, 2 * N).astype(bf16)
        in_maps.append({"wpack": wpack, "wsmall": wsmall, "expB": expB})
    return in_maps


def _unshard(results, bo):
    out = np.empty((B, N, DIM), dtype=np.float32)
    for b in range(B):
        acc = results[2 * b]["outT"].astype(np.float32).sum(axis=0) + results[
            2 * b + 1
        ]["outT"].astype(np.float32).sum(axis=0)
        out[b] = acc.reshape(DIM, N).T + bo[None, :]
    return out


def kernel(x, mask, attn_bias, Wq, Wkv, Wg, bg, Wo, bo):
    """Full inputs in, full output out. mask is all-ones by construction."""
    from concourse.bass_utils import run_bass_kernel_spmd

    x = np.asarray(x, dtype=np.float32)
    attn_bias = np.asarray(attn_bias, dtype=np.float32)
    Wq = np.asarray(Wq, dtype=np.float32)
    Wkv = np.asarray(Wkv, dtype=np.float32)
    Wg = np.asarray(Wg, dtype=np.float32)
    bg = np.asarray(bg, dtype=np.float32)
    Wo = np.asarray(Wo, dtype=np.float32)
    bo = np.asarray(bo, dtype=np.float32)

    if "nc" not in _CACHE:
        _CACHE["nc"] = _build()
    in_maps = _shard_inputs(x, attn_bias, Wq, Wkv, Wg, bg, Wo)
    res = run_bass_kernel_spmd(_CACHE["nc"], in_maps, core_ids=list(range(NCORES)))
    return _unshard(res.results, bo)


# revision 27
# speedup vs baseline: 1.1470x; 1.0075x over previous
"""Trainium2 Bass kernel for nn_Attention (sparse_attention, 8 NeuronCores).

Sharding: data-parallel over batch (4) x tensor-parallel over heads (2 groups
of 4 heads) = 8 cores. Each core computes attention for one batch and 4 heads
entirely in transposed (feature-major) layout, so no on-chip transposes are
needed. exp(attn_bias) is precomputed on the host in bf16, so on-chip softmax
is exp(S) * expB with no PSUM-blocking adds. Wo is row-sharded; each core
returns one bf16 partial per head-pair and the host reduces.

v3: all weights/x in bf16 (halves the weight DMA); DMA issue order tuned so
the first QK starts early and bias chunks stream per (pair, jt); for timing
loops the body is unrolled 2x with double-buffered weight/projection tiles so
iteration n+1's DMA head overlaps iteration n's compute tail.
"""

import os
import sys

for _p in ("/opt/trn_rl_repo", "/root/.axon_site/_ro/trn_rl_repo"):
    if os.path.isdir(_p) and _p not in sys.path:
        sys.path.append(_p)

import numpy as np

B, N, DIM, H, DH = 4, 1024, 512, 8, 64
SCALE = DH**-0.5
HL = 4  # heads per core
HDL = HL * DH  # 256 head-dims per core
NCORES = 8
NJT = N // 128  # 8 key-tiles
NKT = DIM // 128  # 4 contraction tiles

# wpack (bf16) column layout, ordered by first use
_WQK0 = 0  # wq_p0 4kt x 128 | wk_p0 4kt x 128
_XT = 1024  # 4 kt x 1024 tokens
_WV = 5120  # 4 kt x 256
_WQK1 = 6144  # wq_p1 | wk_p1
_WG = 7168  # 4 kt x 256
_WO = 8192  # 2 pair x 512
WPC = 9216

_CACHE = {}


def _build(loop_iters=1):
    import concourse.tile as tile
    from concourse import bacc, mybir

    fp32 = mybir.dt.float32
    f32r = mybir.dt.float32r
    bf16 = mybir.dt.bfloat16

    Exp = mybir.ActivationFunctionType.Exp
    Identity = mybir.ActivationFunctionType.Identity
    mult = mybir.AluOpType.mult

    nc = bacc.Bacc("TRN2", target_bir_lowering=False, debug=False, num_devices=NCORES)

    wpack = nc.dram_tensor("wpack", [128, WPC], bf16, kind="ExternalInput").ap()
    wsmall = nc.dram_tensor("wsmall", [128, 66], f32r, kind="ExternalInput").ap()
    expB = nc.dram_tensor("expB", [2, NJT, 128, 2 * N], bf16, kind="ExternalInput").ap()
    outT = nc.dram_tensor("outT", [2, 4, 128, N], bf16, kind="ExternalOutput").ap()

    from contextlib import ExitStack

    # unroll the loop body 2x so double-buffered tiles rotate across
    # iterations (a hardware loop reuses static SBUF addresses, so a single
    # body would serialize on its weight tiles)
    unroll = 2 if loop_iters > 1 else 1
    assert loop_iters % unroll == 0

    with tile.TileContext(nc) as tc, ExitStack() as stack:
        if loop_iters > 1:
            stack.enter_context(
                tc.For_i(0, loop_iters // unroll, 1, hint_engines=(mybir.EngineType.PE,))
            )
        with (
            tc.tile_pool(name="const", bufs=2) as cpool,
            tc.tile_pool(name="proj", bufs=2) as projpool,
            tc.tile_pool(name="bias", bufs=8) as biaspool,
            tc.tile_pool(name="etile", bufs=4) as epool,
            tc.tile_pool(name="work", bufs=2) as workpool,
            tc.tile_pool(name="psA", bufs=2, space="PSUM") as psA,
            tc.tile_pool(name="psB", bufs=2, space="PSUM") as psB,
        ):
            def body():
                # ---- SBUF homes for weights ----
                wp_sb = cpool.tile([128, WPC], bf16, tag="wp")
                ws_sb = cpool.tile([128, 66], f32r, tag="ws")
                bg_sb = ws_sb[:, 0:2]
                ones_sb = ws_sb[:, 2:66]

                def wq(p, kt):  # [128, 128] stationary for q proj of pair p
                    base = (_WQK0 if p == 0 else _WQK1) + kt * 128
                    return wp_sb[:, base : base + 128]

                def wk(p, kt):
                    base = (_WQK0 if p == 0 else _WQK1) + 512 + kt * 128
                    return wp_sb[:, base : base + 128]

                def xT(kt, lo, size):
                    return wp_sb[:, _XT + kt * 1024 + lo : _XT + kt * 1024 + lo + size]

                def wv(kt):
                    return wp_sb[:, _WV + kt * 256 : _WV + (kt + 1) * 256]

                def wg_(kt, mt):
                    base = _WG + kt * 256 + mt * 128
                    return wp_sb[:, base : base + 128]

                def wo_(p, mt):
                    base = _WO + p * 512 + mt * 128
                    return wp_sb[:, base : base + 128]

                # ---- DMA issue order (single sync ring => priority) ----
                nc.sync.dma_start(ws_sb[:], wsmall)
                # PE p-state prewarm: dummy matmuls off a memset tile keep
                # the PE busy through the DMA head so the projections run at
                # full clock; the Exp LUT preload rides the same window.
                wtile = cpool.tile([1, 512], bf16, tag="wtile")
                nc.vector.memset(wtile[:], 0.5)
                for _ in range(11):
                    pw = psA.tile([1, 512], fp32, tag="big", name="pw")
                    nc.tensor.matmul(pw[:], wtile[0:1, 0:1], wtile[:], start=True, stop=True)
                lutw = cpool.tile([1, 2], fp32, tag="lut")
                nc.scalar.activation(lutw[0:1, 0:1], ones_sb[0:1, 0:1], Exp)

                def wdma(lo, hi):
                    nc.sync.dma_start(wp_sb[:, lo:hi], wpack[:, lo:hi])

                bias_tiles = {}

                def bdma(p, jt):
                    bt = biaspool.tile([128, 2 * N], bf16, tag="bias", name=f"bt{p}_{jt}")
                    for hh in range(2):
                        nc.sync.dma_start(
                            bt[:, hh * N : (hh + 1) * N],
                            expB[p, jt, :, hh * N : (hh + 1) * N],
                        )
                    bias_tiles[(p, jt)] = bt

                wdma(_WQK0, _XT)  # wq_p0 | wk_p0
                for ih in range(2):  # ih-major half-chunks: q0's ih0 matmuls
                    for kt in range(NKT):  # start after half the xT stream
                        lo = _XT + kt * 1024 + ih * 512
                        wdma(lo, lo + 512)
                bdma(0, 0)
                wdma(_WV, _WQK1)
                bdma(0, 1)
                wdma(_WQK1, _WG)
                wdma(_WG, _WO)
                bdma(0, 2)
                wdma(_WO, WPC)
                for jt in range(3, NJT):
                    bdma(0, jt)
                for jt in range(NJT):
                    bdma(1, jt)

                # ---- projections ----
                qT_sb = [projpool.tile([128, N], bf16, tag=f"qT{m}", name=f"qT{m}") for m in range(2)]
                kT_sb = [projpool.tile([128, N], bf16, tag=f"kT{m}", name=f"kT{m}") for m in range(2)]
                gT_sb = [projpool.tile([128, N], fp32, tag=f"gT{m}", name=f"gT{m}") for m in range(2)]

                def projqk(which, p, evac_eng, split=None):
                    """q (which=0) or k (which=1) projection for pair p."""
                    wsel = wq if which == 0 else wk
                    dst = (qT_sb if which == 0 else kT_sb)[p]
                    ps = psA.tile([128, N], fp32, tag="big", name="ps")
                    for kt in range(NKT):
                        lhsT = wsel(p, kt)
                        for ih in range(2):
                            nc.tensor.matmul(
                                ps[:, ih * 512 : ih * 512 + 512],
                                lhsT,
                                xT(kt, ih * 512, 512),
                                start=(kt == 0),
                                stop=(kt == NKT - 1),
                            )
                    cp = nc.scalar.copy if evac_eng == "scalar" else nc.vector.tensor_copy
                    for lo, hi in split or [(0, N)]:
                        cp(dst[:, lo:hi], ps[:, lo:hi])

                def gproj(mt):
                    ps = psA.tile([128, N], fp32, tag="big", name="psg")
                    for kt in range(NKT):
                        lhsT = wg_(kt, mt)
                        for ih in range(2):
                            nc.tensor.matmul(
                                ps[:, ih * 512 : ih * 512 + 512],
                                lhsT,
                                xT(kt, ih * 512, 512),
                                start=(kt == 0),
                                stop=(kt == NKT - 1),
                            )
                    nc.scalar.activation(
                        gT_sb[mt][:], ps[:], Identity, bias=bg_sb[:, mt : mt + 1]
                    )

                # ---- v natural [token, d] + ones column per head (bf16) ----
                vhat_all = projpool.tile([128, NJT * HL * 65], bf16, tag="vhat")
                ones_view = vhat_all[:].rearrange(
                    "p (j h c) -> p j h c", j=NJT, c=65
                )[:, :, :, 64:65]
                nc.vector.memset(ones_view, 1.0)

                def vproj(jt):
                    vv = vhat_all[:, jt * HL * 65 : (jt + 1) * HL * 65].rearrange(
                        "p (h c) -> p h c", h=HL
                    )
                    ps2 = psA.tile([128, HDL], fp32, tag="big", name="ps2")
                    for kt in range(NKT):
                        nc.tensor.matmul(
                            ps2[:],
                            xT(kt, jt * 128, 128),
                            wv(kt),
                            start=(kt == 0),
                            stop=(kt == NKT - 1),
                        )
                    nc.vector.tensor_copy(
                        vv[:, :, 0:64], ps2[:].rearrange("p (h c) -> p h c", h=HL)
                    )

                # split evacs so the first QK (needs qT ih0 + kT cols 0:128)
                # unblocks as early as possible
                projqk(0, 0, "vector", split=[(0, 512), (512, N)])
                projqk(1, 0, "vector", split=[(0, 128), (128, N)])

                # ---- shared state across pairs ----
                U_sb = {}
                ug_sb = [
                    workpool.tile([128, N], bf16, tag=f"ug{p}", name=f"ug{p}", bufs=2)
                    for p in range(2)
                ]
                state = {}

                def attn_pair(p, background):
                    """jt-loop for head-pair p. AV matmuls run 3 (jt, hh)
                    units behind their QK so the in-order PE never waits on
                    the ACT-exp / DVE-mult chain; background thunks fill the
                    remaining PE slack (one slot per unit)."""
                    bgi = iter(background)
                    uv = [
                        psB.tile([65, N], fp32, tag="uv", name=f"uv{p}_{i}")
                        for i in range(2)
                    ]
                    pend = []

                    def flush_av():
                        jt0, hh0, e0 = pend.pop(0)
                        h = 2 * p + hh0
                        base = jt0 * HL * 65 + h * 65
                        for ih in range(2):
                            nc.tensor.matmul(
                                uv[hh0][:, ih * 512 : ih * 512 + 512],
                                vhat_all[:, base : base + 65],
                                e0[:, ih * 512 : ih * 512 + 512],
                                start=(jt0 == 0),
                                stop=(jt0 == NJT - 1),
                            )

                    for jt in range(NJT):
                        bt = bias_tiles[(p, jt)]
                        for hh in range(2):
                            st = psA.tile([128, N], fp32, tag="big", name=f"st{jt}_{hh}")
                            lhsT = kT_sb[p][hh * 64 : hh * 64 + 64, jt * 128 : jt * 128 + 128]
                            for ih in range(2):
                                nc.tensor.matmul(
                                    st[:, ih * 512 : ih * 512 + 512],
                                    lhsT,
                                    qT_sb[p][hh * 64 : hh * 64 + 64, ih * 512 : ih * 512 + 512],
                                    start=True,
                                    stop=True,
                                )
                            e1 = epool.tile([128, N], bf16, tag="e1", name="e1", bufs=3)
                            nc.scalar.activation(e1[:], st[:], Exp)
                            e = epool.tile([128, N], bf16, tag="e", name="e", bufs=5)
                            nc.vector.tensor_tensor(
                                out=e[:],
                                in0=e1[:],
                                in1=bt[:, hh * N : (hh + 1) * N],
                                op=mult,
                            )
                            pend.append((jt, hh, e))
                            if len(pend) > 3:
                                flush_av()
                            th = next(bgi, None)
                            if th is not None:
                                th()
                    for th in bgi:
                        if th is not None:
                            th()
                    return uv, pend, flush_av

                def epi_steps(p, hh, uv):
                    """Divide-by-denominator + gating for (p, hh). For (0,0)
                    the U*gT product runs on GPSIMD in parallel with the
                    reciprocal+broadcast chain (SBUF-SBUF TTs must share a
                    base partition, so only hh==0 qualifies). Elsewhere:
                    gs = broadcast(1/den)*gT (PSUM operand, exempt), then
                    ug = U*gs."""
                    par = (p, hh) == (0, 0)

                    def src():
                        return U_sb[(p, hh)] if p == 0 else uv[hh]

                    def s1():
                        rec = workpool.tile([1, N], f32r, tag="rec", name="rec", bufs=2)
                        with nc.allow_low_precision(reason="feeds PE broadcast"):
                            nc.vector.reciprocal(rec[:], src()[64:65, :])
                        state[("rec", p, hh)] = rec

                    def s1b():
                        ugp = workpool.tile([64, N], fp32, tag="gs", name="ugp", bufs=2)
                        nc.gpsimd.tensor_tensor(
                            out=ugp[:],
                            in0=src()[0:64, :],
                            in1=gT_sb[p][0:64, :],
                            op=mult,
                        )
                        state[("ugp", p, hh)] = ugp

                    def s2():
                        rec = state[("rec", p, hh)]
                        bc = psA.tile([64, N], fp32, tag="big", name="bc")
                        for ih in range(2):
                            nc.tensor.matmul(
                                bc[:, ih * 512 : ih * 512 + 512],
                                ones_sb[0:1, 0:64],
                                rec[0:1, ih * 512 : ih * 512 + 512],
                                start=True,
                                stop=True,
                            )
                        state[("bc", p, hh)] = bc
                        if not par:
                            gs = workpool.tile([64, N], fp32, tag="gs", name="gs", bufs=2)
                            nc.vector.tensor_tensor(
                                out=gs[:],
                                in0=bc[:],
                                in1=gT_sb[p][hh * 64 : hh * 64 + 64, :],
                                op=mult,
                            )
                            state[("gs", p, hh)] = gs

                    def s3():
                        if par:
                            nc.vector.tensor_tensor(
                                out=ug_sb[p][hh * 64 : hh * 64 + 64, :],
                                in0=state[("ugp", p, hh)][:],
                                in1=state[("bc", p, hh)][:],
                                op=mult,
                            )
                        else:
                            nc.vector.tensor_tensor(
                                out=ug_sb[p][hh * 64 : hh * 64 + 64, :],
                                in0=src()[0:64, :],
                                in1=state[("gs", p, hh)][:],
                                op=mult,
                            )

                    return ([s1, s1b, s2, s3] if par else [s1, s2, s3])

                def outproj_step(p, mt):
                    ps = psA.tile([128, N], fp32, tag="big", name="po")
                    lhsT = wo_(p, mt)
                    for ih in range(2):
                        nc.tensor.matmul(
                            ps[:, ih * 512 : ih * 512 + 512],
                            lhsT,
                            ug_sb[p][:, ih * 512 : ih * 512 + 512],
                            start=True,
                            stop=True,
                        )
                    ot = workpool.tile([128, N], bf16, tag="osb", name="osb", bufs=4)
                    if p == 1 or mt % 2 == 0:  # ACT is idle in the tail
                        nc.scalar.copy(ot[:], ps[:])
                    else:
                        nc.vector.tensor_copy(ot[:], ps[:])
                    nc.gpsimd.dma_start(outT[p, mt], ot[:])

                def outproj_steps(p):
                    return [
                        (lambda p=p, mt=mt: outproj_step(p, mt)) for mt in range(4)
                    ]

                # ---- pair 0: backgrounds = v/g projections + pair-1 q/k ----
                bg0 = (
                    [lambda j=j: vproj(j) for j in range(NJT)]
                    + [
                        lambda: projqk(0, 1, "vector"),
                        lambda: projqk(1, 1, "vector"),
                        lambda: gproj(0),
                    ]
                )
                uv0, pend, flush = attn_pair(0, bg0)
                while pend:
                    flush()
                for hh in range(2):
                    U = workpool.tile([65, N], fp32, tag="U", name=f"U0_{hh}", bufs=4)
                    if hh == 0:
                        nc.scalar.copy(U[:], uv0[hh][:])
                    else:
                        nc.vector.tensor_copy(U[:], uv0[hh][:])
                    U_sb[(0, hh)] = U

                # ---- pair 1: backgrounds = pair-0 epilogue/outproj ----
                e00 = epi_steps(0, 0, uv0)
                e01 = epi_steps(0, 1, uv0)
                op0 = outproj_steps(0)
                bg1 = [lambda: gproj(1)] + e00 + [None] + e01 + [None] + op0
                uv1, pend, flush = attn_pair(1, bg1)
                # Tail: drain, then both heads' epilogues with maximal
                # engine parallelism.
                while pend:
                    flush()
                e10 = epi_steps(1, 0, uv1)
                e11 = epi_steps(1, 1, uv1)
                e10[0]()  # recip hh0 (DVE)
                e11[0]()  # recip hh1 (DVE)
                e10[1]()  # broadcast hh0 (PE) + gs hh0 (DVE)
                e11[1]()  # broadcast hh1 (PE) + gs hh1 (DVE)
                e10[2]()  # ug hh0 (DVE)
                e11[2]()  # ug hh1 (DVE)
                for th in outproj_steps(1):
                    th()

            for _ in range(unroll):
                body()

    nc.compile()
    return nc


def _shard_inputs(x, attn_bias, Wq, Wkv, Wg, bg, Wo):
    """Build per-core input maps (host-side layout prep)."""
    import ml_dtypes

    bf16 = ml_dtypes.bfloat16

    def kmaj(w):  # [512, F] -> [128, NKT*F] contraction-tile-major
        f = w.shape[1]
        return np.ascontiguousarray(
            w.reshape(NKT, 128, f).transpose(1, 0, 2)
        ).reshape(128, NKT * f)

    in_maps = []
    for d in range(NCORES):
        b, g = d // 2, d % 2
        cs = slice(g * HDL, (g + 1) * HDL)
        xTh = np.ascontiguousarray(x[b].T)  # [512, 1024]
        wq_all = Wq[:, cs] * SCALE  # [512, 256]
        wk_all = Wkv[:, g * HDL : (g + 1) * HDL]
        wv_all = Wkv[:, H * DH + g * HDL : H * DH + (g + 1) * HDL]
        wg_all = Wg[:, cs]
        chunks = [
            kmaj(wq_all[:, 0:128]),
            kmaj(wk_all[:, 0:128]),
            kmaj(xTh),
            kmaj(wv_all),
            kmaj(wq_all[:, 128:256]),
            kmaj(wk_all[:, 128:256]),
            kmaj(wg_all),
            np.ascontiguousarray(
                Wo[cs, :].reshape(2, 128, DIM).transpose(1, 0, 2).reshape(128, 2 * DIM)
            ),
        ]
        wpack = np.concatenate(chunks, axis=1).astype(bf16)
        wsmall = np.concatenate(
            [
                np.ascontiguousarray(bg[cs].reshape(2, 128).T),
                np.ones((128, 64), np.float32),
            ],
            axis=1,
        ).astype(np.float32)

        ab = attn_bias[b, g * HL : (g + 1) * HL]  # [4, N(i), N(j)]
        abT = ab.transpose(0, 2, 1).reshape(2, 2, NJT, 128, N)  # [p, hh, jt, jpart, i]
        eB = np.exp(abT.transpose(0, 2, 3, 1, 4))  # [p, jt, jpart, hh, i]
        expB = np.ascontiguousarray(eB).reshape(2, NJT, 128, 2 * N).astype(bf16)
        in_maps.append({"wpack": wpack, "wsmall": wsmall, "expB": expB})
    return in_maps


def _unshard(results, bo):
    out = np.empty((B, N, DIM), dtype=np.float32)
    for b in range(B):
        acc = results[2 * b]["outT"].astype(np.float32).sum(axis=0) + results[
            2 * b + 1
        ]["outT"].astype(np.float32).sum(axis=0)
        out[b] = acc.reshape(DIM, N).T + bo[None, :]
    return out


def kernel(x, mask, attn_bias, Wq, Wkv, Wg, bg, Wo, bo):
    """Full inputs in, full output out. mask is all-ones by construction."""
    from concourse.bass_utils import run_bass_kernel_spmd

    x = np.asarray(x, dtype=np.float32)
    attn_bias = np.asarray(attn_bias, dtype=np.float32)
    Wq = np.asarray(Wq, dtype=np.float32)
    Wkv = np.asarray(Wkv, dtype=np.float32)
    Wg = np.asarray(Wg, dtype=np.float32)
    bg = np.asarray(bg, dtype=np.float32)
    Wo = np.asarray(Wo, dtype=np.float32)
    bo = np.asarray(bo, dtype=np.float32)

    if "nc" not in _CACHE:
        _CACHE["nc"] = _build()
    in_maps = _shard_inputs(x, attn_bias, Wq, Wkv, Wg, bg, Wo)
    res = run_bass_kernel_spmd(_CACHE["nc"], in_maps, core_ids=list(range(NCORES)))
    return _unshard(res.results, bo)


# revision 30
# speedup vs baseline: 1.1550x; 1.0069x over previous
"""Trainium2 Bass kernel for nn_Attention (sparse_attention, 8 NeuronCores).

Sharding: data-parallel over batch (4) x tensor-parallel over heads (2 groups
of 4 heads) = 8 cores. Each core computes attention for one batch and 4 heads
entirely in transposed (feature-major) layout, so no on-chip transposes are
needed. exp(attn_bias) is precomputed on the host in bf16, so on-chip softmax
is exp(S) * expB with no PSUM-blocking adds. Wo is row-sharded; each core
returns one bf16 partial per head-pair and the host reduces.

v3: all weights/x in bf16 (halves the weight DMA); DMA issue order tuned so
the first QK starts early and bias chunks stream per (pair, jt, head); the
exp LUT is preloaded and the PE p-state prewarmed during the DMA head; AV
matmuls run 3 units behind their QK so the in-order PE never waits on the
exp/mult chain; the last pair's epilogue interleaves into the AV drain; for
even timing loops the body is unrolled 2x with double-buffered weight and
projection tiles (and output DMAs on the gpsimd ring) so iteration n+1's
DMA head overlaps iteration n's compute tail.
"""

import os
import sys

for _p in ("/opt/trn_rl_repo", "/root/.axon_site/_ro/trn_rl_repo"):
    if os.path.isdir(_p) and _p not in sys.path:
        sys.path.append(_p)

import numpy as np

B, N, DIM, H, DH = 4, 1024, 512, 8, 64
SCALE = DH**-0.5
HL = 4  # heads per core
HDL = HL * DH  # 256 head-dims per core
NCORES = 8
NJT = N // 128  # 8 key-tiles
NKT = DIM // 128  # 4 contraction tiles

# wpack (bf16) column layout, ordered by first use
_WQK0 = 0  # wq_p0 4kt x 128 | wk_p0 4kt x 128
_XT = 1024  # 4 kt x 1024 tokens
_WV = 5120  # 4 kt x 256
_WQK1 = 6144  # wq_p1 | wk_p1
_WG = 7168  # 4 kt x 256
_WO = 8192  # 2 pair x 512
WPC = 9216

_CACHE = {}


def _build(loop_iters=1):
    import concourse.tile as tile
    from concourse import bacc, mybir

    fp32 = mybir.dt.float32
    f32r = mybir.dt.float32r
    bf16 = mybir.dt.bfloat16

    Exp = mybir.ActivationFunctionType.Exp
    Identity = mybir.ActivationFunctionType.Identity
    mult = mybir.AluOpType.mult

    nc = bacc.Bacc("TRN2", target_bir_lowering=False, debug=False, num_devices=NCORES)

    wpack = nc.dram_tensor("wpack", [128, WPC], bf16, kind="ExternalInput").ap()
    wsmall = nc.dram_tensor("wsmall", [128, 66], f32r, kind="ExternalInput").ap()
    expB = nc.dram_tensor("expB", [2, NJT, 128, 2 * N], bf16, kind="ExternalInput").ap()
    outT = nc.dram_tensor("outT", [2, 4, 128, N], bf16, kind="ExternalOutput").ap()

    from contextlib import ExitStack

    # unroll the loop body 2x so double-buffered tiles rotate across
    # iterations (a hardware loop reuses static SBUF addresses, so a single
    # body would serialize on its weight tiles)
    unroll = 2 if (loop_iters > 1 and loop_iters % 2 == 0) else 1

    with tile.TileContext(nc) as tc, ExitStack() as stack:
        if loop_iters > 1:
            stack.enter_context(
                tc.For_i(0, loop_iters // unroll, 1, hint_engines=(mybir.EngineType.PE,))
            )
        with (
            tc.tile_pool(name="const", bufs=2) as cpool,
            tc.tile_pool(name="proj", bufs=2) as projpool,
            tc.tile_pool(name="bias", bufs=8) as biaspool,
            tc.tile_pool(name="etile", bufs=4) as epool,
            tc.tile_pool(name="work", bufs=2) as workpool,
            tc.tile_pool(name="psA", bufs=2, space="PSUM") as psA,
            tc.tile_pool(name="psB", bufs=2, space="PSUM") as psB,
        ):
            def body(first=True):
                # ---- SBUF homes for weights ----
                wp_sb = cpool.tile([128, WPC], bf16, tag="wp")
                ws_sb = cpool.tile([128, 66], f32r, tag="ws")
                bg_sb = ws_sb[:, 0:2]
                ones_sb = ws_sb[:, 2:66]

                def wq(p, kt):  # [128, 128] stationary for q proj of pair p
                    base = (_WQK0 if p == 0 else _WQK1) + kt * 128
                    return wp_sb[:, base : base + 128]

                def wk(p, kt):
                    base = (_WQK0 if p == 0 else _WQK1) + 512 + kt * 128
                    return wp_sb[:, base : base + 128]

                def xT(kt, lo, size):
                    return wp_sb[:, _XT + kt * 1024 + lo : _XT + kt * 1024 + lo + size]

                def wv(kt):
                    return wp_sb[:, _WV + kt * 256 : _WV + (kt + 1) * 256]

                def wg_(kt, mt):
                    base = _WG + kt * 256 + mt * 128
                    return wp_sb[:, base : base + 128]

                def wo_(p, mt):
                    base = _WO + p * 512 + mt * 128
                    return wp_sb[:, base : base + 128]

                # ---- DMA issue order (single sync ring => priority) ----
                nc.sync.dma_start(ws_sb[:], wsmall)
                # PE p-state prewarm: dummy matmuls off a memset tile keep
                # the PE busy through the DMA head so the projections run at
                # full clock; the Exp LUT preload rides the same window.
                if first:
                    wtile = cpool.tile([1, 512], bf16, tag="wtile")
                    nc.vector.memset(wtile[:], 0.5)
                    for _ in range(11):
                        pw = psA.tile([1, 512], fp32, tag="big", name="pw")
                        nc.tensor.matmul(pw[:], wtile[0:1, 0:1], wtile[:], start=True, stop=True)
                lutw = cpool.tile([1, 2], fp32, tag="lut")
                nc.scalar.activation(lutw[0:1, 0:1], ones_sb[0:1, 0:1], Exp)

                def wdma(lo, hi):
                    nc.sync.dma_start(wp_sb[:, lo:hi], wpack[:, lo:hi])

                bias_tiles = {}

                def bdma(p, jt, split=False):
                    bt = biaspool.tile([128, 2 * N], bf16, tag="bias", name=f"bt{p}_{jt}")
                    ring = nc.gpsimd if p == 1 else nc.sync
                    if split:
                        for hh in range(2):
                            ring.dma_start(
                                bt[:, hh * N : (hh + 1) * N],
                                expB[p, jt, :, hh * N : (hh + 1) * N],
                            )
                    else:
                        ring.dma_start(bt[:], expB[p, jt])
                    bias_tiles[(p, jt)] = bt

                wdma(_WQK0, _XT)  # wq_p0 | wk_p0
                for ih in range(2):  # ih-major half-chunks: q0's ih0 matmuls
                    for kt in range(NKT):  # start after half the xT stream
                        lo = _XT + kt * 1024 + ih * 512
                        wdma(lo, lo + 512)
                bdma(0, 0, split=True)
                wdma(_WV, _WQK1)
                bdma(0, 1, split=True)
                wdma(_WQK1, _WG)
                wdma(_WG, _WO)
                bdma(0, 2)
                wdma(_WO, WPC)
                for jt in range(3, NJT):
                    bdma(0, jt)
                for jt in range(NJT):
                    bdma(1, jt)

                # ---- projections ----
                qT_sb = [projpool.tile([128, N], bf16, tag=f"qT{m}", name=f"qT{m}") for m in range(2)]
                kT_sb = [projpool.tile([128, N], bf16, tag=f"kT{m}", name=f"kT{m}") for m in range(2)]
                gT_sb = [projpool.tile([128, N], fp32, tag=f"gT{m}", name=f"gT{m}") for m in range(2)]

                def projqk(which, p, evac_eng, split=None):
                    """q (which=0) or k (which=1) projection for pair p."""
                    wsel = wq if which == 0 else wk
                    dst = (qT_sb if which == 0 else kT_sb)[p]
                    ps = psA.tile([128, N], fp32, tag="big", name="ps")
                    for kt in range(NKT):
                        lhsT = wsel(p, kt)
                        for ih in range(2):
                            nc.tensor.matmul(
                                ps[:, ih * 512 : ih * 512 + 512],
                                lhsT,
                                xT(kt, ih * 512, 512),
                                start=(kt == 0),
                                stop=(kt == NKT - 1),
                            )
                    cp = nc.scalar.copy if evac_eng == "scalar" else nc.vector.tensor_copy
                    for lo, hi in split or [(0, N)]:
                        cp(dst[:, lo:hi], ps[:, lo:hi])

                def gproj(mt):
                    ps = psA.tile([128, N], fp32, tag="big", name="psg")
                    for kt in range(NKT):
                        lhsT = wg_(kt, mt)
                        for ih in range(2):
                            nc.tensor.matmul(
                                ps[:, ih * 512 : ih * 512 + 512],
                                lhsT,
                                xT(kt, ih * 512, 512),
                                start=(kt == 0),
                                stop=(kt == NKT - 1),
                            )
                    nc.scalar.activation(
                        gT_sb[mt][:], ps[:], Identity, bias=bg_sb[:, mt : mt + 1]
                    )

                # ---- v natural [token, d] + ones column per head (bf16) ----
                vhat_all = projpool.tile([128, NJT * HL * 65], bf16, tag="vhat")
                ones_view = vhat_all[:].rearrange(
                    "p (j h c) -> p j h c", j=NJT, c=65
                )[:, :, :, 64:65]
                nc.vector.memset(ones_view, 1.0)

                def vproj(jt):
                    vv = vhat_all[:, jt * HL * 65 : (jt + 1) * HL * 65].rearrange(
                        "p (h c) -> p h c", h=HL
                    )
                    ps2 = psA.tile([128, HDL], fp32, tag="big", name="ps2")
                    for kt in range(NKT):
                        nc.tensor.matmul(
                            ps2[:],
                            xT(kt, jt * 128, 128),
                            wv(kt),
                            start=(kt == 0),
                            stop=(kt == NKT - 1),
                        )
                    nc.vector.tensor_copy(
                        vv[:, :, 0:64], ps2[:].rearrange("p (h c) -> p h c", h=HL)
                    )

                # split evacs so the first QK (needs qT ih0 + kT cols 0:128)
                # unblocks as early as possible
                projqk(0, 0, "vector", split=[(0, 512), (512, N)])
                projqk(1, 0, "vector", split=[(0, 128), (128, N)])

                # ---- shared state across pairs ----
                U_sb = {}
                ug_sb = [
                    workpool.tile([128, N], bf16, tag=f"ug{p}", name=f"ug{p}", bufs=2)
                    for p in range(2)
                ]
                state = {}

                def attn_pair(p, background):
                    """jt-loop for head-pair p. AV matmuls run 3 (jt, hh)
                    units behind their QK so the in-order PE never waits on
                    the ACT-exp / DVE-mult chain; background thunks fill the
                    remaining PE slack (one slot per unit)."""
                    bgi = iter(background)
                    uv = [
                        psB.tile([65, N], fp32, tag="uv", name=f"uv{p}_{i}")
                        for i in range(2)
                    ]
                    pend = []

                    def flush_av():
                        jt0, hh0, e0 = pend.pop(0)
                        h = 2 * p + hh0
                        base = jt0 * HL * 65 + h * 65
                        for ih in range(2):
                            nc.tensor.matmul(
                                uv[hh0][:, ih * 512 : ih * 512 + 512],
                                vhat_all[:, base : base + 65],
                                e0[:, ih * 512 : ih * 512 + 512],
                                start=(jt0 == 0),
                                stop=(jt0 == NJT - 1),
                            )

                    for jt in range(NJT):
                        bt = bias_tiles[(p, jt)]
                        for hh in range(2):
                            st = psA.tile([128, N], fp32, tag="big", name=f"st{jt}_{hh}")
                            lhsT = kT_sb[p][hh * 64 : hh * 64 + 64, jt * 128 : jt * 128 + 128]
                            for ih in range(2):
                                nc.tensor.matmul(
                                    st[:, ih * 512 : ih * 512 + 512],
                                    lhsT,
                                    qT_sb[p][hh * 64 : hh * 64 + 64, ih * 512 : ih * 512 + 512],
                                    start=True,
                                    stop=True,
                                )
                            e1 = epool.tile([128, N], bf16, tag="e1", name="e1", bufs=3)
                            nc.scalar.activation(e1[:], st[:], Exp)
                            e = epool.tile([128, N], bf16, tag="e", name="e", bufs=5)
                            nc.vector.tensor_tensor(
                                out=e[:],
                                in0=e1[:],
                                in1=bt[:, hh * N : (hh + 1) * N],
                                op=mult,
                            )
                            pend.append((jt, hh, e))
                            if len(pend) > 3:
                                flush_av()
                            th = next(bgi, None)
                            if th is not None:
                                th()
                    for th in bgi:
                        if th is not None:
                            th()
                    return uv, pend, flush_av

                def epi_steps(p, hh, uv):
                    """Divide-by-denominator + gating for (p, hh). For (0,0)
                    the U*gT product runs on GPSIMD in parallel with the
                    reciprocal+broadcast chain (SBUF-SBUF TTs must share a
                    base partition, so only hh==0 qualifies). Elsewhere:
                    gs = broadcast(1/den)*gT (PSUM operand, exempt), then
                    ug = U*gs."""
                    par = (p, hh) == (0, 0)

                    def src():
                        return U_sb[(p, hh)] if p == 0 else uv[hh]

                    def s1():
                        rec = workpool.tile([1, N], f32r, tag="rec", name="rec", bufs=2)
                        with nc.allow_low_precision(reason="feeds PE broadcast"):
                            nc.vector.reciprocal(rec[:], src()[64:65, :])
                        state[("rec", p, hh)] = rec

                    def s1b():
                        ugp = workpool.tile([64, N], fp32, tag="gs", name="ugp", bufs=2)
                        nc.gpsimd.tensor_tensor(
                            out=ugp[:],
                            in0=src()[0:64, :],
                            in1=gT_sb[p][0:64, :],
                            op=mult,
                        )
                        state[("ugp", p, hh)] = ugp

                    def s2():
                        rec = state[("rec", p, hh)]
                        bc = psA.tile([64, N], fp32, tag="big", name="bc")
                        for ih in range(2):
                            nc.tensor.matmul(
                                bc[:, ih * 512 : ih * 512 + 512],
                                ones_sb[0:1, 0:64],
                                rec[0:1, ih * 512 : ih * 512 + 512],
                                start=True,
                                stop=True,
                            )
                        state[("bc", p, hh)] = bc
                        if not par:
                            gs = workpool.tile([64, N], fp32, tag="gs", name="gs", bufs=2)
                            nc.vector.tensor_tensor(
                                out=gs[:],
                                in0=bc[:],
                                in1=gT_sb[p][hh * 64 : hh * 64 + 64, :],
                                op=mult,
                            )
                            state[("gs", p, hh)] = gs

                    def s3():
                        if par:
                            nc.vector.tensor_tensor(
                                out=ug_sb[p][hh * 64 : hh * 64 + 64, :],
                                in0=state[("ugp", p, hh)][:],
                                in1=state[("bc", p, hh)][:],
                                op=mult,
                            )
                        else:
                            nc.vector.tensor_tensor(
                                out=ug_sb[p][hh * 64 : hh * 64 + 64, :],
                                in0=src()[0:64, :],
                                in1=state[("gs", p, hh)][:],
                                op=mult,
                            )

                    return ([s1, s1b, s2, s3] if par else [s1, s2, s3])

                def outproj_step(p, mt):
                    ps = psA.tile([128, N], fp32, tag="big", name="po")
                    lhsT = wo_(p, mt)
                    for ih in range(2):
                        nc.tensor.matmul(
                            ps[:, ih * 512 : ih * 512 + 512],
                            lhsT,
                            ug_sb[p][:, ih * 512 : ih * 512 + 512],
                            start=True,
                            stop=True,
                        )
                    ot = workpool.tile([128, N], bf16, tag="osb", name="osb", bufs=4)
                    if p == 1 or mt % 2 == 0:  # ACT is idle in the tail
                        nc.scalar.copy(ot[:], ps[:])
                    else:
                        nc.vector.tensor_copy(ot[:], ps[:])
                    nc.gpsimd.dma_start(outT[p, mt], ot[:])

                def outproj_steps(p):
                    return [
                        (lambda p=p, mt=mt: outproj_step(p, mt)) for mt in range(4)
                    ]

                # ---- pair 0: backgrounds = v/g projections + pair-1 q/k ----
                bg0 = (
                    [lambda j=j: vproj(j) for j in range(NJT)]
                    + [
                        lambda: projqk(0, 1, "vector"),
                        lambda: projqk(1, 1, "vector"),
                        lambda: gproj(0),
                    ]
                )
                uv0, pend, flush = attn_pair(0, bg0)
                while pend:
                    flush()
                for hh in range(2):
                    U = workpool.tile([65, N], fp32, tag="U", name=f"U0_{hh}", bufs=4)
                    if hh == 0:
                        nc.scalar.copy(U[:], uv0[hh][:])
                    else:
                        nc.vector.tensor_copy(U[:], uv0[hh][:])
                    U_sb[(0, hh)] = U

                # ---- pair 1: backgrounds = pair-0 epilogue/outproj ----
                e00 = epi_steps(0, 0, uv0)
                e01 = epi_steps(0, 1, uv0)
                op0 = outproj_steps(0)
                bg1 = [lambda: gproj(1)] + e00 + [None] + e01 + [None] + op0
                uv1, pend, flush = attn_pair(1, bg1)
                # Tail: drain, then both heads' epilogues with maximal
                # engine parallelism.
                while pend:
                    flush()
                e10 = epi_steps(1, 0, uv1)
                e11 = epi_steps(1, 1, uv1)
                e10[0]()  # recip hh0 (DVE)
                e11[0]()  # recip hh1 (DVE)
                e10[1]()  # broadcast hh0 (PE) + gs hh0 (DVE)
                e11[1]()  # broadcast hh1 (PE) + gs hh1 (DVE)
                e10[2]()  # ug hh0 (DVE)
                e11[2]()  # ug hh1 (DVE)
                for th in outproj_steps(1):
                    th()

            for i in range(unroll):
                body(first=(i == 0))

    nc.compile()
    return nc


def _shard_inputs(x, attn_bias, Wq, Wkv, Wg, bg, Wo):
    """Build per-core input maps (host-side layout prep)."""
    import ml_dtypes

    bf16 = ml_dtypes.bfloat16

    def kmaj(w):  # [512, F] -> [128, NKT*F] contraction-tile-major
        f = w.shape[1]
        return np.ascontiguousarray(
            w.reshape(NKT, 128, f).transpose(1, 0, 2)
        ).reshape(128, NKT * f)

    in_maps = []
    for d in range(NCORES):
        b, g = d // 2, d % 2
        cs = slice(g * HDL, (g + 1) * HDL)
        xTh = np.ascontiguousarray(x[b].T)  # [512, 1024]
        wq_all = Wq[:, cs] * SCALE  # [512, 256]
        wk_all = Wkv[:, g * HDL : (g + 1) * HDL]
        wv_all = Wkv[:, H * DH + g * HDL : H * DH + (g + 1) * HDL]
        wg_all = Wg[:, cs]
        chunks = [
            kmaj(wq_all[:, 0:128]),
            kmaj(wk_all[:, 0:128]),
            kmaj(xTh),
            kmaj(wv_all),
            kmaj(wq_all[:, 128:256]),
            kmaj(wk_all[:, 128:256]),
            kmaj(wg_all),
            np.ascontiguousarray(
                Wo[cs, :].reshape(2, 128, DIM).transpose(1, 0, 2).reshape(128, 2 * DIM)
            ),
        ]
        wpack = np.concatenate(chunks, axis=1).astype(bf16)
        wsmall = np.concatenate(
            [
                np.ascontiguousarray(bg[cs].reshape(2, 128).T),
                np.ones((128, 64), np.float32),
            ],
            axis=1,
        ).astype(np.float32)

        ab = attn_bias[b, g * HL : (g + 1) * HL]  # [4, N(i), N(j)]
        abT = ab.transpose(0, 2, 1).reshape(2, 2, NJT, 128, N)  # [p, hh, jt, jpart, i]
        eB = np.exp(abT.transpose(0, 2, 3, 1, 4))  # [p, jt, jpart, hh, i]
        expB = np.ascontiguousarray(eB).reshape(2, NJT, 128, 2 * N).astype(bf16)
        in_maps.append({"wpack": wpack, "wsmall": wsmall, "expB": expB})
    return in_maps


def _unshard(results, bo):
    out = np.empty((B, N, DIM), dtype=np.float32)
    for b in range(B):
        acc = results[2 * b]["outT"].astype(np.float32).sum(axis=0) + results[
            2 * b + 1
        ]["outT"].astype(np.float32).sum(axis=0)
        out[b] = acc.reshape(DIM, N).T + bo[None, :]
    return out


def kernel(x, mask, attn_bias, Wq, Wkv, Wg, bg, Wo, bo):
    """Full inputs in, full output out. mask is all-ones by construction."""
    from concourse.bass_utils import run_bass_kernel_spmd

    x = np.asarray(x, dtype=np.float32)
    attn_bias = np.asarray(attn_bias, dtype=np.float32)
    Wq = np.asarray(Wq, dtype=np.float32)
    Wkv = np.asarray(Wkv, dtype=np.float32)
    Wg = np.asarray(Wg, dtype=np.float32)
    bg = np.asarray(bg, dtype=np.float32)
    Wo = np.asarray(Wo, dtype=np.float32)
    bo = np.asarray(bo, dtype=np.float32)

    if "nc" not in _CACHE:
        _CACHE["nc"] = _build()
    in_maps = _shard_inputs(x, attn_bias, Wq, Wkv, Wg, bg, Wo)
    res = run_bass_kernel_spmd(_CACHE["nc"], in_maps, core_ids=list(range(NCORES)))
    return _unshard(res.results, bo)


# revision 31
# speedup vs baseline: 1.1922x; 1.0322x over previous
"""Trainium2 Bass kernel for nn_Attention (sparse_attention, 8 NeuronCores).

Sharding: data-parallel over batch (4) x tensor-parallel over heads (2 groups
of 4 heads) = 8 cores. Each core computes attention for one batch and 4 heads
entirely in transposed (feature-major) layout, so no on-chip transposes are
needed. exp(attn_bias) is precomputed on the host in bf16, so on-chip softmax
is exp(S) * expB with no PSUM-blocking adds. Wo is row-sharded; each core
returns one bf16 partial per head-pair and the host reduces.

v3: all weights/x in bf16 (halves the weight DMA); DMA issue order tuned so
the first QK starts early and bias chunks stream per (pair, jt, head); the
exp LUT is preloaded and the PE p-state prewarmed during the DMA head; AV
matmuls run 3 units behind their QK so the in-order PE never waits on the
exp/mult chain; the last pair's epilogue interleaves into the AV drain; for
even timing loops the body is unrolled 2x with double-buffered weight and
projection tiles (and output DMAs on the gpsimd ring) so iteration n+1's
DMA head overlaps iteration n's compute tail.
"""

import os
import sys

for _p in ("/opt/trn_rl_repo", "/root/.axon_site/_ro/trn_rl_repo"):
    if os.path.isdir(_p) and _p not in sys.path:
        sys.path.append(_p)

import numpy as np

B, N, DIM, H, DH = 4, 1024, 512, 8, 64
SCALE = DH**-0.5
HL = 4  # heads per core
HDL = HL * DH  # 256 head-dims per core
NCORES = 8
NJT = N // 128  # 8 key-tiles
NKT = DIM // 128  # 4 contraction tiles

# wpack (bf16) column layout, ordered by first use
_WQK0 = 0  # wq_p0 4kt x 128 | wk_p0 4kt x 128
_XT = 1024  # 4 kt x 1024 tokens
_WV = 5120  # 4 kt x 256
_WQK1 = 6144  # wq_p1 | wk_p1
_WG = 7168  # 4 kt x 256
_WO = 8192  # 2 pair x 512
WPC = 9216

_CACHE = {}


def _build(loop_iters=1):
    import concourse.tile as tile
    from concourse import bacc, mybir

    fp32 = mybir.dt.float32
    f32r = mybir.dt.float32r
    bf16 = mybir.dt.bfloat16

    Exp = mybir.ActivationFunctionType.Exp
    Identity = mybir.ActivationFunctionType.Identity
    mult = mybir.AluOpType.mult

    nc = bacc.Bacc("TRN2", target_bir_lowering=False, debug=False, num_devices=NCORES)

    wpack = nc.dram_tensor("wpack", [128, WPC], bf16, kind="ExternalInput").ap()
    wsmall = nc.dram_tensor("wsmall", [128, 66], f32r, kind="ExternalInput").ap()
    expB = nc.dram_tensor("expB", [2, NJT, 128, 2 * N], bf16, kind="ExternalInput").ap()
    outT = nc.dram_tensor("outT", [2, 4, 128, N], bf16, kind="ExternalOutput").ap()

    from contextlib import ExitStack

    # unroll the loop body 2x so double-buffered tiles rotate across
    # iterations (a hardware loop reuses static SBUF addresses, so a single
    # body would serialize on its weight tiles)
    unroll = 2 if (loop_iters > 1 and loop_iters % 2 == 0) else 1

    with tile.TileContext(nc) as tc, ExitStack() as stack:
        if loop_iters > 1:
            stack.enter_context(
                tc.For_i(0, loop_iters // unroll, 1, hint_engines=(mybir.EngineType.PE, mybir.EngineType.Activation, mybir.EngineType.DVE, mybir.EngineType.SP, mybir.EngineType.Pool))
            )
        with (
            tc.tile_pool(name="const", bufs=2) as cpool,
            tc.tile_pool(name="proj", bufs=2) as projpool,
            tc.tile_pool(name="bias", bufs=10) as biaspool,
            tc.tile_pool(name="etile", bufs=4) as epool,
            tc.tile_pool(name="work", bufs=2) as workpool,
            tc.tile_pool(name="psA", bufs=2, space="PSUM") as psA,
            tc.tile_pool(name="psB", bufs=2, space="PSUM") as psB,
        ):
            def body(first=True):
                # ---- SBUF homes for weights ----
                wp_sb = cpool.tile([128, WPC], bf16, tag="wp")
                ws_sb = cpool.tile([128, 66], f32r, tag="ws")
                bg_sb = ws_sb[:, 0:2]
                ones_sb = ws_sb[:, 2:66]

                def wq(p, kt):  # [128, 128] stationary for q proj of pair p
                    base = (_WQK0 if p == 0 else _WQK1) + kt * 128
                    return wp_sb[:, base : base + 128]

                def wk(p, kt):
                    base = (_WQK0 if p == 0 else _WQK1) + 512 + kt * 128
                    return wp_sb[:, base : base + 128]

                def xT(kt, lo, size):
                    return wp_sb[:, _XT + kt * 1024 + lo : _XT + kt * 1024 + lo + size]

                def wv(kt):
                    return wp_sb[:, _WV + kt * 256 : _WV + (kt + 1) * 256]

                def wg_(kt, mt):
                    base = _WG + kt * 256 + mt * 128
                    return wp_sb[:, base : base + 128]

                def wo_(p, mt):
                    base = _WO + p * 512 + mt * 128
                    return wp_sb[:, base : base + 128]

                # ---- DMA issue order (single sync ring => priority) ----
                nc.sync.dma_start(ws_sb[:], wsmall)
                # PE p-state prewarm: dummy matmuls off a memset tile keep
                # the PE busy through the DMA head so the projections run at
                # full clock; the Exp LUT preload rides the same window.
                if first:
                    wtile = cpool.tile([1, 512], bf16, tag="wtile")
                    nc.vector.memset(wtile[:], 0.5)
                    for _ in range(11):
                        pw = psA.tile([1, 512], fp32, tag="big", name="pw")
                        nc.tensor.matmul(pw[:], wtile[0:1, 0:1], wtile[:], start=True, stop=True)
                lutw = cpool.tile([1, 2], fp32, tag="lut")
                nc.scalar.activation(lutw[0:1, 0:1], ones_sb[0:1, 0:1], Exp)

                def wdma(lo, hi):
                    nc.sync.dma_start(wp_sb[:, lo:hi], wpack[:, lo:hi])

                bias_tiles = {}

                def bdma(p, jt, split=False):
                    bt = biaspool.tile([128, 2 * N], bf16, tag="bias", name=f"bt{p}_{jt}")
                    ring = nc.gpsimd if p == 1 else nc.sync
                    if split:
                        for hh in range(2):
                            ring.dma_start(
                                bt[:, hh * N : (hh + 1) * N],
                                expB[p, jt, :, hh * N : (hh + 1) * N],
                            )
                    else:
                        ring.dma_start(bt[:], expB[p, jt])
                    bias_tiles[(p, jt)] = bt

                wdma(_WQK0, _XT)  # wq_p0 | wk_p0
                for ih in range(2):  # ih-major half-chunks: q0's ih0 matmuls
                    for kt in range(NKT):  # start after half the xT stream
                        lo = _XT + kt * 1024 + ih * 512
                        wdma(lo, lo + 512)
                bdma(0, 0, split=True)
                wdma(_WV, _WQK1)
                bdma(0, 1, split=True)
                wdma(_WQK1, _WG)
                wdma(_WG, _WO)
                bdma(0, 2)
                wdma(_WO, WPC)
                for jt in range(3, NJT):
                    bdma(0, jt)
                for jt in range(NJT):
                    bdma(1, jt)

                # ---- projections ----
                qT_sb = [projpool.tile([128, N], bf16, tag=f"qT{m}", name=f"qT{m}") for m in range(2)]
                kT_sb = [projpool.tile([128, N], bf16, tag=f"kT{m}", name=f"kT{m}") for m in range(2)]
                gT_sb = [projpool.tile([128, N], fp32, tag=f"gT{m}", name=f"gT{m}") for m in range(2)]

                def projqk(which, p, evac_eng, split=None):
                    """q (which=0) or k (which=1) projection for pair p."""
                    wsel = wq if which == 0 else wk
                    dst = (qT_sb if which == 0 else kT_sb)[p]
                    ps = psA.tile([128, N], fp32, tag="big", name="ps")
                    for kt in range(NKT):
                        lhsT = wsel(p, kt)
                        for ih in range(2):
                            nc.tensor.matmul(
                                ps[:, ih * 512 : ih * 512 + 512],
                                lhsT,
                                xT(kt, ih * 512, 512),
                                start=(kt == 0),
                                stop=(kt == NKT - 1),
                            )
                    cp = nc.scalar.copy if evac_eng == "scalar" else nc.vector.tensor_copy
                    for lo, hi in split or [(0, N)]:
                        cp(dst[:, lo:hi], ps[:, lo:hi])

                def gproj(mt):
                    ps = psA.tile([128, N], fp32, tag="big", name="psg")
                    for kt in range(NKT):
                        lhsT = wg_(kt, mt)
                        for ih in range(2):
                            nc.tensor.matmul(
                                ps[:, ih * 512 : ih * 512 + 512],
                                lhsT,
                                xT(kt, ih * 512, 512),
                                start=(kt == 0),
                                stop=(kt == NKT - 1),
                            )
                    nc.scalar.activation(
                        gT_sb[mt][:], ps[:], Identity, bias=bg_sb[:, mt : mt + 1]
                    )

                # ---- v natural [token, d] + ones column per head (bf16) ----
                vhat_all = projpool.tile([128, NJT * HL * 65], bf16, tag="vhat")
                ones_view = vhat_all[:].rearrange(
                    "p (j h c) -> p j h c", j=NJT, c=65
                )[:, :, :, 64:65]
                nc.vector.memset(ones_view, 1.0)

                def vproj(jt):
                    vv = vhat_all[:, jt * HL * 65 : (jt + 1) * HL * 65].rearrange(
                        "p (h c) -> p h c", h=HL
                    )
                    ps2 = psA.tile([128, HDL], fp32, tag="big", name="ps2")
                    for kt in range(NKT):
                        nc.tensor.matmul(
                            ps2[:],
                            xT(kt, jt * 128, 128),
                            wv(kt),
                            start=(kt == 0),
                            stop=(kt == NKT - 1),
                        )
                    nc.vector.tensor_copy(
                        vv[:, :, 0:64], ps2[:].rearrange("p (h c) -> p h c", h=HL)
                    )

                # split evacs so the first QK (needs qT ih0 + kT cols 0:128)
                # unblocks as early as possible
                projqk(0, 0, "vector", split=[(0, 512), (512, N)])
                projqk(1, 0, "vector", split=[(0, 128), (128, N)])

                # ---- shared state across pairs ----
                U_sb = {}
                ug_sb = [
                    workpool.tile([128, N], bf16, tag=f"ug{p}", name=f"ug{p}", bufs=2)
                    for p in range(2)
                ]
                state = {}

                def attn_pair(p, background):
                    """jt-loop for head-pair p. AV matmuls run 3 (jt, hh)
                    units behind their QK so the in-order PE never waits on
                    the ACT-exp / DVE-mult chain; background thunks fill the
                    remaining PE slack (one slot per unit)."""
                    bgi = iter(background)
                    uv = [
                        psB.tile([65, N], fp32, tag="uv", name=f"uv{p}_{i}")
                        for i in range(2)
                    ]
                    pend = []

                    def flush_av():
                        jt0, hh0, e0 = pend.pop(0)
                        h = 2 * p + hh0
                        base = jt0 * HL * 65 + h * 65
                        for ih in range(2):
                            nc.tensor.matmul(
                                uv[hh0][:, ih * 512 : ih * 512 + 512],
                                vhat_all[:, base : base + 65],
                                e0[:, ih * 512 : ih * 512 + 512],
                                start=(jt0 == 0),
                                stop=(jt0 == NJT - 1),
                            )

                    for jt in range(NJT):
                        bt = bias_tiles[(p, jt)]
                        for hh in range(2):
                            st = psA.tile([128, N], fp32, tag="big", name=f"st{jt}_{hh}")
                            lhsT = kT_sb[p][hh * 64 : hh * 64 + 64, jt * 128 : jt * 128 + 128]
                            for ih in range(2):
                                nc.tensor.matmul(
                                    st[:, ih * 512 : ih * 512 + 512],
                                    lhsT,
                                    qT_sb[p][hh * 64 : hh * 64 + 64, ih * 512 : ih * 512 + 512],
                                    start=True,
                                    stop=True,
                                )
                            e1 = epool.tile([128, N], bf16, tag="e1", name="e1", bufs=3)
                            nc.scalar.activation(e1[:], st[:], Exp)
                            e = epool.tile([128, N], bf16, tag="e", name="e", bufs=6)
                            nc.vector.tensor_tensor(
                                out=e[:],
                                in0=e1[:],
                                in1=bt[:, hh * N : (hh + 1) * N],
                                op=mult,
                            )
                            pend.append((jt, hh, e))
                            if len(pend) > 4:
                                flush_av()
                            th = next(bgi, None)
                            if th is not None:
                                th()
                    for th in bgi:
                        if th is not None:
                            th()
                    return uv, pend, flush_av

                def epi_steps(p, hh, uv):
                    """Divide-by-denominator + gating for (p, hh). For (0,0)
                    the U*gT product runs on GPSIMD in parallel with the
                    reciprocal+broadcast chain (SBUF-SBUF TTs must share a
                    base partition, so only hh==0 qualifies). Elsewhere:
                    gs = broadcast(1/den)*gT (PSUM operand, exempt), then
                    ug = U*gs."""
                    par = (p, hh) == (0, 0)

                    def src():
                        return U_sb[(p, hh)] if p == 0 else uv[hh]

                    def s1():
                        rec = workpool.tile([1, N], f32r, tag="rec", name="rec", bufs=2)
                        with nc.allow_low_precision(reason="feeds PE broadcast"):
                            nc.vector.reciprocal(rec[:], src()[64:65, :])
                        state[("rec", p, hh)] = rec

                    def s1b():
                        ugp = workpool.tile([64, N], fp32, tag="gs", name="ugp", bufs=2)
                        nc.gpsimd.tensor_tensor(
                            out=ugp[:],
                            in0=src()[0:64, :],
                            in1=gT_sb[p][0:64, :],
                            op=mult,
                        )
                        state[("ugp", p, hh)] = ugp

                    def s2():
                        rec = state[("rec", p, hh)]
                        bc = psA.tile([64, N], fp32, tag="big", name="bc")
                        for ih in range(2):
                            nc.tensor.matmul(
                                bc[:, ih * 512 : ih * 512 + 512],
                                ones_sb[0:1, 0:64],
                                rec[0:1, ih * 512 : ih * 512 + 512],
                                start=True,
                                stop=True,
                            )
                        state[("bc", p, hh)] = bc
                        if not par:
                            gs = workpool.tile([64, N], fp32, tag="gs", name="gs", bufs=2)
                            nc.vector.tensor_tensor(
                                out=gs[:],
                                in0=bc[:],
                                in1=gT_sb[p][hh * 64 : hh * 64 + 64, :],
                                op=mult,
                            )
                            state[("gs", p, hh)] = gs

                    def s3():
                        if par:
                            nc.vector.tensor_tensor(
                                out=ug_sb[p][hh * 64 : hh * 64 + 64, :],
                                in0=state[("ugp", p, hh)][:],
                                in1=state[("bc", p, hh)][:],
                                op=mult,
                            )
                        else:
                            nc.vector.tensor_tensor(
                                out=ug_sb[p][hh * 64 : hh * 64 + 64, :],
                                in0=src()[0:64, :],
                                in1=state[("gs", p, hh)][:],
                                op=mult,
                            )

                    return ([s1, s1b, s2, s3] if par else [s1, s2, s3])

                def outproj_step(p, mt):
                    ps = psA.tile([128, N], fp32, tag="big", name="po")
                    lhsT = wo_(p, mt)
                    for ih in range(2):
                        nc.tensor.matmul(
                            ps[:, ih * 512 : ih * 512 + 512],
                            lhsT,
                            ug_sb[p][:, ih * 512 : ih * 512 + 512],
                            start=True,
                            stop=True,
                        )
                    ot = workpool.tile([128, N], bf16, tag="osb", name="osb", bufs=4)
                    if p == 1 or mt % 2 == 0:  # ACT is idle in the tail
                        nc.scalar.copy(ot[:], ps[:])
                    else:
                        nc.vector.tensor_copy(ot[:], ps[:])
                    nc.gpsimd.dma_start(outT[p, mt], ot[:])

                def outproj_steps(p):
                    return [
                        (lambda p=p, mt=mt: outproj_step(p, mt)) for mt in range(4)
                    ]

                # ---- pair 0: backgrounds = v/g projections + pair-1 q/k ----
                bg0 = (
                    [lambda j=j: vproj(j) for j in range(NJT)]
                    + [
                        lambda: projqk(0, 1, "vector"),
                        lambda: projqk(1, 1, "vector"),
                        lambda: gproj(0),
                    ]
                )
                uv0, pend, flush = attn_pair(0, bg0)
                while pend:
                    flush()
                for hh in range(2):
                    U = workpool.tile([65, N], fp32, tag="U", name=f"U0_{hh}", bufs=4)
                    if hh == 0:
                        nc.scalar.copy(U[:], uv0[hh][:])
                    else:
                        nc.vector.tensor_copy(U[:], uv0[hh][:])
                    U_sb[(0, hh)] = U

                # ---- pair 1: backgrounds = pair-0 epilogue/outproj ----
                e00 = epi_steps(0, 0, uv0)
                e01 = epi_steps(0, 1, uv0)
                op0 = outproj_steps(0)
                bg1 = [lambda: gproj(1)] + e00 + [None] + e01 + [None] + op0
                uv1, pend, flush = attn_pair(1, bg1)
                # Tail: drain, then both heads' epilogues with maximal
                # engine parallelism.
                while pend:
                    flush()
                e10 = epi_steps(1, 0, uv1)
                e11 = epi_steps(1, 1, uv1)
                e10[0]()  # recip hh0 (DVE)
                e11[0]()  # recip hh1 (DVE)
                e10[1]()  # broadcast hh0 (PE) + gs hh0 (DVE)
                e11[1]()  # broadcast hh1 (PE) + gs hh1 (DVE)
                e10[2]()  # ug hh0 (DVE)
                e11[2]()  # ug hh1 (DVE)
                for th in outproj_steps(1):
                    th()

            for i in range(unroll):
                body(first=(i == 0))

    nc.compile()
    return nc


def _shard_inputs(x, attn_bias, Wq, Wkv, Wg, bg, Wo):
    """Build per-core input maps (host-side layout prep)."""
    import ml_dtypes

    bf16 = ml_dtypes.bfloat16

    def kmaj(w):  # [512, F] -> [128, NKT*F] contraction-tile-major
        f = w.shape[1]
        return np.ascontiguousarray(
            w.reshape(NKT, 128, f).transpose(1, 0, 2)
        ).reshape(128, NKT * f)

    in_maps = []
    for d in range(NCORES):
        b, g = d // 2, d % 2
        cs = slice(g * HDL, (g + 1) * HDL)
        xTh = np.ascontiguousarray(x[b].T)  # [512, 1024]
        wq_all = Wq[:, cs] * SCALE  # [512, 256]
        wk_all = Wkv[:, g * HDL : (g + 1) * HDL]
        wv_all = Wkv[:, H * DH + g * HDL : H * DH + (g + 1) * HDL]
        wg_all = Wg[:, cs]
        chunks = [
            kmaj(wq_all[:, 0:128]),
            kmaj(wk_all[:, 0:128]),
            kmaj(xTh),
            kmaj(wv_all),
            kmaj(wq_all[:, 128:256]),
            kmaj(wk_all[:, 128:256]),
            kmaj(wg_all),
            np.ascontiguousarray(
                Wo[cs, :].reshape(2, 128, DIM).transpose(1, 0, 2).reshape(128, 2 * DIM)
            ),
        ]
        wpack = np.concatenate(chunks, axis=1).astype(bf16)
        wsmall = np.concatenate(
            [
                np.ascontiguousarray(bg[cs].reshape(2, 128).T),
                np.ones((128, 64), np.float32),
            ],
            axis=1,
        ).astype(np.float32)

        ab = attn_bias[b, g * HL : (g + 1) * HL]  # [4, N(i), N(j)]
        abT = ab.transpose(0, 2, 1).reshape(2, 2, NJT, 128, N)  # [p, hh, jt, jpart, i]
        eB = np.exp(abT.transpose(0, 2, 3, 1, 4))  # [p, jt, jpart, hh, i]
        expB = np.ascontiguousarray(eB).reshape(2, NJT, 128, 2 * N).astype(bf16)
        in_maps.append({"wpack": wpack, "wsmall": wsmall, "expB": expB})
    return in_maps


def _unshard(results, bo):
    out = np.empty((B, N, DIM), dtype=np.float32)
    for b in range(B):
        acc = results[2 * b]["outT"].astype(np.float32).sum(axis=0) + results[
            2 * b + 1
        ]["outT"].astype(np.float32).sum(axis=0)
        out[b] = acc.reshape(DIM, N).T + bo[None, :]
    return out


def kernel(x, mask, attn_bias, Wq, Wkv, Wg, bg, Wo, bo):
    """Full inputs in, full output out. mask is all-ones by construction."""
    from concourse.bass_utils import run_bass_kernel_spmd

    x = np.asarray(x, dtype=np.float32)
    attn_bias = np.asarray(attn_bias, dtype=np.float32)
    Wq = np.asarray(Wq, dtype=np.float32)
    Wkv = np.asarray(Wkv, dtype=np.float32)
    Wg = np.asarray(Wg, dtype=np.float32)
    bg = np.asarray(bg, dtype=np.float32)
    Wo = np.asarray(Wo, dtype=np.float32)
    bo = np.asarray(bo, dtype=np.float32)

    if "nc" not in _CACHE:
        _CACHE["nc"] = _build()
    in_maps = _shard_inputs(x, attn_bias, Wq, Wkv, Wg, bg, Wo)
    res = run_bass_kernel_spmd(_CACHE["nc"], in_maps, core_ids=list(range(NCORES)))
    return _unshard(res.results, bo)
